# revision 7
# baseline (speedup 1.0000x reference)
"""Transformer decoder block (self-attn + cross-attn + FFN, post-LN) on 8
Trainium2 NeuronCores.

v3: head-sharded causal self-attention + token-sharded cross-attn/FFN.

8 cores = 2 batches x 4 ranks. Rank j of a batch group:
  - SA: computes heads [4j, 4j+4) for ALL 2048 tokens. K/V/Q projected
    locally from the full decoder input (no collective before attention).
    Causal structure is uniform across cores: per 512-query chunk qc only
    key tiles kt < 4(qc+1) are computed (62.5% of the full score work) and
    only the 4 diagonal tiles are masked. WO partials [2048, 1024] are
    staged in fp8e3 and summed across the 4 ranks with two column-split
    ReduceScatters; rank j receives its own 512-token slice.
  - CA: token-sharded as v2 — each rank projects K/V for its 512 encoder
    tokens, one fused AllGather per tensor (fp8e3), attention for its 512
    queries over all 2048 keys. The CA AllGathers are issued early and fly
    under the SA compute.
  - FFN + all residual/LN paths: token-sharded (512 tokens per rank).

v3 micro-optimizations vs v2:
  - CA score/AV matmuls read the fp8 AllGather buffers directly as the
    stationary operand (mixed fp8xbf16 matmul) — the fp8->bf16 DVE casts
    are gone.
  - K/V transport in fp8e3 (e3m4) instead of e4m3: halves the
    quantization error of the collective path.
  - softmax 1/Z via reciprocal_approx_fast (~5x faster than reciprocal).
  - Z-accumulation split between the DVE and the (otherwise idle) GpSimd
    engine: two partial accumulators, merged by the partition-sum matmul.
  - residual 1 (decoder input + bv@wo+bo) precomputed host-side and DMAd
    in [token, feature] layout directly (no PE transposes to rebuild it).
All matmuls bf16 (or fp8e3 stationary) with fp32 PSUM accumulation.
"""

from contextlib import ExitStack

import numpy as np
import ml_dtypes

import concourse.bass as bass
import concourse.bacc as bacc
import concourse.mybir as mybir
import concourse.tile as tile
from concourse import bass_utils
from concourse.masks import make_identity

BF16 = mybir.dt.bfloat16
F8E3 = mybir.dt.float8e3
F32 = mybir.dt.float32
AF = mybir.ActivationFunctionType
OP = mybir.AluOpType

B, S, D, H, F = 2, 2048, 1024, 16, 4096
DH = 64
EPS = 1e-5
CH = 512          # output tokens per core
DT = D // 128     # 8 feature tiles
NKT = S // 128    # 16 key tiles
NMT = CH // 128   # 4 token tiles per core (output)
NFT = F // 128    # 32 FFN hidden tiles
NST = S // 128    # 16 token tiles (full sequence)

_CACHED = None


def build():
    nc = bacc.Bacc("TRN2", target_bir_lowering=False, debug=False,
                   enable_asserts=False, num_devices=8)

    # ---- per-core DRAM I/O ----
    d_xT = nc.dram_tensor("xT", [D, S], BF16, kind="ExternalInput")
    d_eTq = nc.dram_tensor("eTq", [D, CH], BF16, kind="ExternalInput")
    d_res1b = nc.dram_tensor("res1b", [CH, D], BF16, kind="ExternalInput")
    d_wqh = nc.dram_tensor("sa_wq_h", [D, 256], BF16, kind="ExternalInput")
    d_wkh = nc.dram_tensor("sa_wk_h", [D, 256], BF16, kind="ExternalInput")
    d_wvh = nc.dram_tensor("sa_wv_h", [D, 256], BF16, kind="ExternalInput")
    d_woh = nc.dram_tensor("sa_wo_h", [256, D], BF16, kind="ExternalInput")
    d_bqh = nc.dram_tensor("sa_bq_h", [256], F32, kind="ExternalInput")
    d_bkh = nc.dram_tensor("sa_bk_h", [256], F32, kind="ExternalInput")
    cnames = ["ca_wq", "ca_wk", "ca_wv", "ca_wo"]
    d_w = {n: nc.dram_tensor(n, [D, D], BF16, kind="ExternalInput")
           for n in cnames}
    d_w1 = nc.dram_tensor("f_w1", [D, F], BF16, kind="ExternalInput")
    d_w2 = nc.dram_tensor("f_w2", [F, D], BF16, kind="ExternalInput")
    d_bq_ca = nc.dram_tensor("ca_bq", [D], F32, kind="ExternalInput")
    d_bk_ca = nc.dram_tensor("ca_bk", [D], F32, kind="ExternalInput")
    d_b1 = nc.dram_tensor("f_b1", [F], F32, kind="ExternalInput")
    d_cvec = nc.dram_tensor("cvec", [D], BF16, kind="ExternalInput")
    d_b2v = nc.dram_tensor("b2v", [D], BF16, kind="ExternalInput")
    d_gbt = {n: nc.dram_tensor(n, [D], BF16, kind="ExternalInput")
             for n in ["sa_g", "sa_bt", "ca_g", "ca_bt", "f_g", "f_bt"]}
    d_out = nc.dram_tensor("out", [CH, D], F32, kind="ExternalOutput")

    cc = {
        "kt_in": nc.dram_tensor("cc_ca_kt_in", [D, CH], F8E3, kind="Internal"),
        "kt_out": nc.dram_tensor("cc_ca_kt_out", [4 * D, CH], F8E3,
                                 kind="Internal"),
        "v_in": nc.dram_tensor("cc_ca_v_in", [CH, D], F8E3, kind="Internal"),
        "v_out": nc.dram_tensor("cc_ca_v_out", [S, D], F8E3, kind="Internal"),
        "rs_in0": nc.dram_tensor("cc_rs_in0", [S, 512], F8E3, kind="Internal"),
        "rs_in1": nc.dram_tensor("cc_rs_in1", [S, 512], F8E3, kind="Internal"),
        "rs_out0": nc.dram_tensor("cc_rs_out0", [CH, 512], F8E3,
                                  kind="Internal"),
        "rs_out1": nc.dram_tensor("cc_rs_out1", [CH, 512], F8E3,
                                  kind="Internal"),
    }
    GROUPS = [[0, 1, 2, 3], [4, 5, 6, 7]]

    with tile.TileContext(nc) as tc, ExitStack() as ctx:
        const = ctx.enter_context(tc.tile_pool(name="const", bufs=1))
        wpool = ctx.enter_context(tc.tile_pool(name="wpool", bufs=8))
        qpool = ctx.enter_context(tc.tile_pool(name="qpool", bufs=16))
        resp = ctx.enter_context(tc.tile_pool(name="resp", bufs=8))
        scrp = ctx.enter_context(tc.tile_pool(name="scrp", bufs=2))
        ps_s = ctx.enter_context(tc.tile_pool(name="ps_s", bufs=2, space="PSUM"))
        ps_av = ctx.enter_context(tc.tile_pool(name="ps_av", bufs=2, space="PSUM"))
        ps_m = ctx.enter_context(tc.tile_pool(name="ps_m", bufs=3, space="PSUM"))

        ident = const.tile([128, 128], F32, tag="ident")
        make_identity(nc, ident)
        identb = const.tile([128, 128], BF16, tag="identb")
        nc.vector.tensor_copy(identb, ident)
        onescol = const.tile([128, 1], BF16, tag="onescol")
        nc.vector.memset(onescol, 1.0)
        onesrow = const.tile([1, 64], F32, tag="onesrow")
        nc.vector.memset(onesrow, 1.0)
        epst = const.tile([128, 1], F32, tag="epst")
        nc.vector.memset(epst, EPS)
        zerot = const.tile([128, 1], F32, tag="zerot")
        nc.vector.memset(zerot, 0.0)

        def bias_cols(dram, ntiles, name):
            t = const.tile([128, ntiles], F32, tag=name, name=name)
            src = bass.AP(tensor=dram.ap().tensor, offset=0,
                          ap=[[1, 128], [128, ntiles]])
            nc.sync.dma_start(out=t, in_=src)
            return t

        def bcast_row(dram, tag, name):
            t = const.tile([128, D], BF16, tag=tag, bufs=2, name=name)
            src = bass.AP(tensor=dram.ap().tensor, offset=0, ap=[[0, 128], [1, D]])
            nc.sync.dma_start(out=t, in_=src)
            return t

        bq_h = bias_cols(d_bqh, 2, "bqh")
        bk_h = bias_cols(d_bkh, 2, "bkh")
        bq_ca = bias_cols(d_bq_ca, DT, "bqca")
        bk_ca = bias_cols(d_bk_ca, DT, "bkca")
        b1c = bias_cols(d_b1, NFT, "b1c")

        def layer_norm(src, g_t, bt_t, out):
            """[128, D] f32 LN along free dim; out may alias src."""
            stats = scrp.tile([128, 2, 6], F32, tag="lnstat", name="lnstat")
            for s in range(2):
                nc.vector.bn_stats(out=stats[:, s, :],
                                   in_=src[:, s * 512:(s + 1) * 512])
            mv = scrp.tile([128, 2], F32, tag="lnmv", name="lnmv")
            nc.vector.bn_aggr(out=mv, in_=stats)
            rstd = scrp.tile([128, 1], F32, tag="lnrstd", name="lnrstd")
            nc.scalar.activation(out=rstd, in_=mv[:, 1:2], func=AF.Sqrt,
                                 bias=epst, scale=1.0)
            nc.vector.reciprocal(out=rstd, in_=rstd)
            cent = scrp.tile([128, D], F32, tag="scr", name="cent")
            nc.vector.scalar_tensor_tensor(out=cent, in0=src, scalar=mv[:, 0:1],
                                           in1=g_t, op0=OP.subtract, op1=OP.mult)
            nc.vector.scalar_tensor_tensor(out=out, in0=cent, scalar=rstd,
                                           in1=bt_t, op0=OP.mult, op1=OP.add)

        def load_w8(wd, ncols=D):
            ws = []
            for k in range(DT):
                t = wpool.tile([128, ncols], BF16, tag="w", name=f"w_{k}")
                nc.sync.dma_start(out=t, in_=wd.ap()[k * 128:(k + 1) * 128, :])
                ws.append(t)
            return ws

        # =================== CA K/V local + AllGathers (first) ============
        with nc.named_scope("ca_kvlocal"):
            eq = []
            for k in range(DT):
                t = qpool.tile([128, CH], BF16, tag="qt", name=f"eq{k}")
                nc.sync.dma_start(out=t, in_=d_eTq.ap()[k * 128:(k + 1) * 128, :])
                eq.append(t)
            wk = load_w8(d_w["ca_wk"])
            for m in range(DT):
                ps = ps_m.tile([128, CH], F32, tag="ps_m", name="lkps")
                for k in range(DT):
                    nc.tensor.matmul(ps, wk[k][:, m * 128:(m + 1) * 128],
                                     eq[k], start=(k == 0), stop=(k == DT - 1))
                st = scrp.tile([128, CH], F8E3, tag="stage", bufs=4, name="ktst")
                nc.scalar.activation(out=st, in_=ps, func=AF.Identity,
                                     bias=bk_ca[:, m:m + 1], scale=1.0)
                nc.sync.dma_start(
                    out=cc["kt_in"].ap()[m * 128:(m + 1) * 128, :], in_=st)
            nc.gpsimd.collective_compute(
                "AllGather", mybir.AluOpType.bypass,
                ins=[cc["kt_in"].ap()], outs=[cc["kt_out"].ap()],
                replica_groups=GROUPS)
            wv = load_w8(d_w["ca_wv"])
            for tt in range(NMT):
                for n in range(2):
                    ps = ps_m.tile([128, CH], F32, tag="ps_m", name="lvps")
                    for k in range(DT):
                        nc.tensor.matmul(
                            ps, eq[k][:, tt * 128:(tt + 1) * 128],
                            wv[k][:, n * 512:(n + 1) * 512],
                            start=(k == 0), stop=(k == DT - 1))
                    st = scrp.tile([128, CH], F8E3, tag="stage", bufs=4,
                                   name="vst")
                    nc.scalar.activation(out=st, in_=ps, func=AF.Copy)
                    nc.sync.dma_start(
                        out=cc["v_in"].ap()[tt * 128:(tt + 1) * 128,
                                            n * 512:(n + 1) * 512],
                        in_=st)
            nc.gpsimd.collective_compute(
                "AllGather", mybir.AluOpType.bypass,
                ins=[cc["v_in"].ap()], outs=[cc["v_out"].ap()],
                replica_groups=GROUPS)

        # =================== SA local projections =========================
        with ExitStack() as sa_ctx:
            big = sa_ctx.enter_context(tc.tile_pool(name="big", bufs=2))
            maskp = sa_ctx.enter_context(tc.tile_pool(name="maskp", bufs=1))
            ppool = sa_ctx.enter_context(tc.tile_pool(name="ppool", bufs=3))
            zpool = sa_ctx.enter_context(tc.tile_pool(name="zpool", bufs=2))
            zsm = sa_ctx.enter_context(tc.tile_pool(name="zsm", bufs=2))

            # K^T and Q^T: [128 (2 heads x 64 dh), 2048 tokens] per pair
            KT = [big.tile([128, S], BF16, tag="KT", bufs=2, name=f"KT{g}")
                  for g in range(2)]
            QT = [big.tile([128, S], BF16, tag="QT", bufs=2, name=f"QT{g}")
                  for g in range(2)]
            # V: [128 tokens per tile, 16 tiles, 256 dh]  (bias folded)
            vt = big.tile([128, NST, 256], F8E3, tag="vt", bufs=1, name="vt")

            with ExitStack() as proj_ctx, nc.named_scope("sa_proj"):
                projp = proj_ctx.enter_context(
                    tc.tile_pool(name="projp", bufs=8))
                xt = []
                for k in range(DT):
                    t = projp.tile([128, S], BF16, tag="xt", bufs=8,
                                   name=f"xt{k}")
                    nc.sync.dma_start(out=t,
                                      in_=d_xT.ap()[k * 128:(k + 1) * 128, :])
                    xt.append(t)

                def load_wh(wd, tag):
                    ws = []
                    for k in range(DT):
                        t = projp.tile([128, 256], BF16, tag=tag, bufs=8,
                                       name=f"{tag}{k}")
                        nc.sync.dma_start(
                            out=t, in_=wd.ap()[k * 128:(k + 1) * 128, :])
                        ws.append(t)
                    return ws

                wqh = load_wh(d_wqh, "wqh")
                wkh = load_wh(d_wkh, "wkh")
                wvh = load_wh(d_wvh, "wvh")

                for g in range(2):
                    for tc4 in range(4):
                        ps = ps_m.tile([128, CH], F32, tag="ps_m", name="kps")
                        for k in range(DT):
                            nc.tensor.matmul(
                                ps, wkh[k][:, g * 128:(g + 1) * 128],
                                xt[k][:, tc4 * 512:(tc4 + 1) * 512],
                                start=(k == 0), stop=(k == DT - 1))
                        nc.scalar.activation(
                            out=KT[g][:, tc4 * 512:(tc4 + 1) * 512], in_=ps,
                            func=AF.Identity, bias=bk_h[:, g:g + 1], scale=1.0)
                for g in range(2):
                    for tc4 in range(4):
                        ps = ps_m.tile([128, CH], F32, tag="ps_m", name="qps")
                        for k in range(DT):
                            nc.tensor.matmul(
                                ps, wqh[k][:, g * 128:(g + 1) * 128],
                                xt[k][:, tc4 * 512:(tc4 + 1) * 512],
                                start=(k == 0), stop=(k == DT - 1))
                        nc.scalar.activation(
                            out=QT[g][:, tc4 * 512:(tc4 + 1) * 512], in_=ps,
                            func=AF.Identity, bias=bq_h[:, g:g + 1], scale=1.0)
                for tt in range(NST):
                    ps = ps_m.tile([128, 256], F32, tag="ps_m", name="vps")
                    for k in range(DT):
                        nc.tensor.matmul(
                            ps, xt[k][:, tt * 128:(tt + 1) * 128], wvh[k],
                            start=(k == 0), stop=(k == DT - 1))
                    nc.scalar.activation(out=vt[:, tt, :], in_=ps,
                                         func=AF.Copy)

            # ---- causal mask for the 4 diagonal tiles (same on all cores):
            # amask[k, dkt, q] = -240 if (128*dkt + k > q) else 0
            qmk = scrp.tile([128, CH], F32, tag="qmk", bufs=1, name="qmk")
            nc.gpsimd.iota(qmk, pattern=[[1, CH]], base=0,
                           channel_multiplier=-1,
                           allow_small_or_imprecise_dtypes=True)
            amask = maskp.tile([128, 4, CH], BF16, tag="mask", name="amask")
            for dkt in range(4):
                nc.vector.tensor_scalar(out=amask[:, dkt, :], in0=qmk,
                                        scalar1=float(128 * dkt),
                                        scalar2=-240.0, op0=OP.is_lt,
                                        op1=OP.mult)

            # =================== SA attention pairs =======================
            attnT = [big.tile([128, S], BF16, tag="atn", bufs=2,
                              name=f"atn{g}") for g in range(2)]
            GPN = {0: 0, 1: 1, 2: 1, 3: 2}   # trailing kt2 iters on GpSimd
            for g in range(2):
                for qc in range(4):
                    with nc.named_scope(f"sa_g{g}q{qc}"):
                        nkt = 4 * qc + 4
                        nkt2 = nkt // 2
                        ngp = GPN[qc]
                        ndv = nkt2 - ngp
                        qa = QT[g][0:64, qc * 512:(qc + 1) * 512]
                        qb = QT[g][64:128, qc * 512:(qc + 1) * 512]
                        pav = ps_av.tile([128, CH], F32, tag="ps_av", bufs=1,
                                         name="pav")
                        zacc_d = zpool.tile([128, 2 * CH], BF16, tag="zd",
                                            bufs=2, name="zacc_d")
                        zacc_g = zpool.tile([128, 2 * CH], BF16, tag="zg",
                                            bufs=2, name="zacc_g")
                        for kt2 in range(nkt2):
                            pt2 = ppool.tile([128, 2, 2 * CH], BF16, tag="pt",
                                             name="pt")
                            for sub in range(2):
                                kt = 2 * kt2 + sub
                                diag = kt >= 4 * qc
                                pss = ps_s.tile([128, 2 * CH], F32, tag="ps_s",
                                                name="pss")
                                ksl = KT[g][:, kt * 128:(kt + 1) * 128]
                                nc.tensor.matmul(pss[:, 0:CH], ksl[0:64, :],
                                                 qa, start=True,
                                                 stop=not diag)
                                nc.tensor.matmul(pss[:, CH:2 * CH],
                                                 ksl[64:128, :], qb,
                                                 start=True, stop=True)
                                if diag:
                                    dkt = kt - 4 * qc
                                    nc.tensor.matmul(pss[:, 0:CH], identb,
                                                     amask[:, dkt, :],
                                                     start=False, stop=True,
                                                     skip_group_check=True)
                                nc.scalar.activation(
                                    out=pt2[:, sub, :], in_=pss, func=AF.Exp,
                                    bias=zerot, scale=1.0 / np.sqrt(DH))
                                if diag:
                                    dkt = kt - 4 * qc
                                    nc.vector.scalar_tensor_tensor(
                                        out=pt2[:, sub, CH:2 * CH],
                                        in0=amask[:, dkt, :],
                                        scalar=-1.0, op0=OP.is_ge,
                                        in1=pt2[:, sub, CH:2 * CH],
                                        op1=OP.mult)
                            # Z accumulation: first ndv iters on DVE, rest gp
                            if kt2 < ndv:
                                eng, zt, first = nc.vector, zacc_d, kt2 == 0
                            else:
                                eng, zt, first = nc.gpsimd, zacc_g, kt2 == ndv
                            if first:
                                eng.tensor_copy(zt, pt2[:, 0, :])
                            else:
                                eng.tensor_add(zt, zt, pt2[:, 0, :])
                            eng.tensor_add(zt, zt, pt2[:, 1, :])
                            # AV accumulate
                            for sub in range(2):
                                kt = 2 * kt2 + sub
                                vsl = vt[:, kt, g * 128:(g + 1) * 128]
                                nc.tensor.matmul(pav[0:64, :], vsl[:, 0:64],
                                                 pt2[:, sub, 0:CH],
                                                 start=(kt == 0),
                                                 stop=(kt == nkt - 1))
                                nc.tensor.matmul(pav[64:128, :],
                                                 vsl[:, 64:128],
                                                 pt2[:, sub, CH:2 * CH],
                                                 start=(kt == 0),
                                                 stop=(kt == nkt - 1))
                        at = attnT[g][:, qc * 512:(qc + 1) * 512]
                        nc.vector.tensor_copy(at, pav)
                        # Z: partition-sum of both accumulators, broadcast,
                        # approx-reciprocal, normalize.
                        zs = zsm.tile([1, 2 * CH], F32, tag="zs", bufs=2,
                                      name="zs")
                        for h2 in range(2):
                            zf = ps_m.tile([1, CH], F32, tag="ps_m", name="zf")
                            nc.tensor.matmul(zf, onescol,
                                             zacc_d[:, h2 * CH:(h2 + 1) * CH],
                                             start=True, stop=(ngp == 0))
                            if ngp:
                                nc.tensor.matmul(
                                    zf, onescol,
                                    zacc_g[:, h2 * CH:(h2 + 1) * CH],
                                    start=False, stop=True)
                            nc.vector.tensor_copy(zs[:, h2 * CH:(h2 + 1) * CH],
                                                  zf)
                        przU = ps_m.tile([128, CH], F32, tag="ps_m",
                                         name="przU")
                        nc.tensor.matmul(przU[0:64, :], onesrow[0:1, :],
                                         zs[:, 0:CH], start=True, stop=True,
                                         tile_position=(0, 0))
                        nc.tensor.matmul(przU[64:128, :], onesrow[0:1, :],
                                         zs[:, CH:2 * CH], start=True,
                                         stop=True, tile_position=(0, 64))
                        rec = zsm.tile([128, CH], F32, tag="rec", bufs=2,
                                       name="rec")
                        nc.vector.reciprocal_approx_fast(out=rec, in_=przU)
                        nc.vector.tensor_mul(at, at, rec)

            # =================== SA WO + ReduceScatter ====================
            with nc.named_scope("sa_wo_rs"):
                woh = [big.tile([128, D], BF16, tag="woh", bufs=2,
                                name=f"woh{g}") for g in range(2)]
                for g in range(2):
                    nc.sync.dma_start(
                        out=woh[g], in_=d_woh.ap()[g * 128:(g + 1) * 128, :])
                for n in range(2):
                    for mt in range(NST):
                        ps = ps_m.tile([128, CH], F32, tag="ps_m", name="wops")
                        for g in range(2):
                            nc.tensor.matmul(
                                ps, attnT[g][:, mt * 128:(mt + 1) * 128],
                                woh[g][:, n * 512:(n + 1) * 512],
                                start=(g == 0), stop=(g == 1))
                        st = scrp.tile([128, CH], F8E3, tag="stage", bufs=4,
                                       name="rsst")
                        if mt % 2 == 0:
                            nc.scalar.activation(out=st, in_=ps, func=AF.Copy)
                        else:
                            nc.vector.tensor_copy(st, ps)
                        nc.sync.dma_start(
                            out=cc[f"rs_in{n}"].ap()[mt * 128:(mt + 1) * 128,
                                                     :],
                            in_=st)
                    nc.gpsimd.collective_compute(
                        "ReduceScatter", mybir.AluOpType.add,
                        ins=[cc[f"rs_in{n}"].ap()],
                        outs=[cc[f"rs_out{n}"].ap()],
                        replica_groups=GROUPS)

        # =================== post-RS: x1 = LN(RS + res1b) =================
        def transposeT(x_tiles, out_tag):
            """4 [128, D] f32 token-major -> 8 [128, CH] bf16 feature-major."""
            outs = [qpool.tile([128, CH], BF16, tag="qt",
                               name=f"{out_tag}{i}") for i in range(DT)]
            for mt in range(NMT):
                for ft in range(DT):
                    pst = ps_m.tile([128, 128], F32, tag="ps_m", name="tps")
                    nc.tensor.transpose(
                        pst, x_tiles[mt][:, ft * 128:(ft + 1) * 128], ident)
                    nc.vector.tensor_copy(
                        outs[ft][:, mt * 128:(mt + 1) * 128], pst)
            return outs

        with nc.named_scope("sa_ln"):
            g1 = bcast_row(d_gbt["sa_g"], "gt", "g1")
            bt1 = bcast_row(d_gbt["sa_bt"], "gt", "bt1")
            x1 = []
            for mt in range(NMT):
                rst = scrp.tile([128, D], F8E3, tag="rst", bufs=2, name="rst")
                for n in range(2):
                    nc.sync.dma_start(
                        out=rst[:, n * 512:(n + 1) * 512],
                        in_=cc[f"rs_out{n}"].ap()[mt * 128:(mt + 1) * 128, :])
                r1t = scrp.tile([128, D], BF16, tag="r1t", bufs=2, name="r1t")
                nc.sync.dma_start(
                    out=r1t, in_=d_res1b.ap()[mt * 128:(mt + 1) * 128, :])
                pre = resp.tile([128, D], F32, tag="persist", name=f"x1_{mt}")
                nc.vector.tensor_add(pre, rst, r1t)
                layer_norm(pre, g1, bt1, pre)
                x1.append(pre)
            x1T = transposeT(x1, "x1T")

        # =================== CA attention =================================
        with ExitStack() as ca_ctx:
            kvp = ca_ctx.enter_context(tc.tile_pool(name="kvp", bufs=2))
            vpp = ca_ctx.enter_context(tc.tile_pool(name="vpp", bufs=1))
            ppool2 = ca_ctx.enter_context(tc.tile_pool(name="ppool2", bufs=4))
            zpool2 = ca_ctx.enter_context(tc.tile_pool(name="zpool2", bufs=2))
            attp = ca_ctx.enter_context(tc.tile_pool(name="attp", bufs=8))
            zsm2 = ca_ctx.enter_context(tc.tile_pool(name="zsm2", bufs=2))

            with nc.named_scope("ca_q"):
                wqc = load_w8(d_w["ca_wq"])
                QTca = []
                for m in range(DT):
                    ps = ps_m.tile([128, CH], F32, tag="ps_m", name="cqps")
                    for k in range(DT):
                        nc.tensor.matmul(ps, wqc[k][:, m * 128:(m + 1) * 128],
                                         x1T[k], start=(k == 0),
                                         stop=(k == DT - 1))
                    o = qpool.tile([128, CH], BF16, tag="qt", name=f"qca{m}")
                    nc.scalar.activation(out=o, in_=ps, func=AF.Identity,
                                         bias=bq_ca[:, m:m + 1], scale=1.0)
                    QTca.append(o)

            aun = []
            vcur = None
            for hp in range(H // 2):
                with nc.named_scope(f"ca_pair{hp}"):
                    # K^T for this pair straight from the fp8e3 AG buffer
                    ktp = kvp.tile([128, 4, CH], F8E3, tag="ktp", name="ktp")
                    nc.sync.dma_start(
                        out=ktp,
                        in_=bass.AP(tensor=cc["kt_out"].ap().tensor,
                                    offset=128 * hp * CH,
                                    ap=[[CH, 128], [D * CH, 4], [1, CH]]))
                    ktb = ktp.rearrange("p a q -> p (a q)")
                    if hp % 2 == 0:
                        vte = vpp.tile([128, NKT, 256], F8E3, tag="vpp",
                                       bufs=1, name="vpp")
                        nc.sync.dma_start(
                            out=vte,
                            in_=bass.AP(tensor=cc["v_out"].ap().tensor,
                                        offset=(hp // 2) * 256,
                                        ap=[[D, 128], [128 * D, NKT],
                                            [1, 256]]))
                        vcur = vte
                    voff = (hp % 2) * 128

                    qa = QTca[hp][0:64, :]
                    qb = QTca[hp][64:128, :]
                    pav = ps_av.tile([128, CH], F32, tag="ps_av", bufs=1,
                                     name="cpav")
                    zacc_d = zpool2.tile([128, 2 * CH], BF16, tag="zd",
                                         bufs=2, name="czd")
                    zacc_g = zpool2.tile([128, 2 * CH], BF16, tag="zg",
                                         bufs=2, name="czg")
                    NDV = 5   # kt2 0..4 on DVE, 5..7 on GpSimd
                    for kt2 in range(NKT // 2):
                        pt2 = ppool2.tile([128, 2, 2 * CH], BF16, tag="pt",
                                          name="cpt")
                        for sub in range(2):
                            kt = 2 * kt2 + sub
                            pss = ps_s.tile([128, 2 * CH], F32, tag="ps_s",
                                            name="cpss")
                            ksl = ktb[:, kt * 128:(kt + 1) * 128]
                            nc.tensor.matmul(pss[:, 0:CH], ksl[0:64, :], qa,
                                             start=True, stop=True)
                            nc.tensor.matmul(pss[:, CH:2 * CH],
                                             ksl[64:128, :], qb,
                                             start=True, stop=True)
                            nc.scalar.activation(
                                out=pt2[:, sub, :], in_=pss, func=AF.Exp,
                                bias=zerot, scale=1.0 / np.sqrt(DH))
                        if kt2 < NDV:
                            eng, zt, first = nc.vector, zacc_d, kt2 == 0
                        else:
                            eng, zt, first = nc.gpsimd, zacc_g, kt2 == NDV
                        if first:
                            eng.tensor_copy(zt, pt2[:, 0, :])
                        else:
                            eng.tensor_add(zt, zt, pt2[:, 0, :])
                        eng.tensor_add(zt, zt, pt2[:, 1, :])
                        for sub in range(2):
                            kt = 2 * kt2 + sub
                            vsl = vcur[:, kt, voff:voff + 128]
                            nc.tensor.matmul(pav[0:64, :], vsl[:, 0:64],
                                             pt2[:, sub, 0:CH],
                                             start=(kt == 0),
                                             stop=(kt == NKT - 1))
                            nc.tensor.matmul(pav[64:128, :], vsl[:, 64:128],
                                             pt2[:, sub, CH:2 * CH],
                                             start=(kt == 0),
                                             stop=(kt == NKT - 1))
                    at = attp.tile([128, CH], BF16, tag="aun", name=f"aun{hp}")
                    nc.vector.tensor_copy(at, pav)
                    aun.append(at)
                    zs = zsm2.tile([1, 2 * CH], F32, tag="zs", bufs=2,
                                   name="czs")
                    for h2 in range(2):
                        zf = ps_m.tile([1, CH], F32, tag="ps_m", name="czf")
                        nc.tensor.matmul(zf, onescol,
                                         zacc_d[:, h2 * CH:(h2 + 1) * CH],
                                         start=True, stop=False)
                        nc.tensor.matmul(zf, onescol,
                                         zacc_g[:, h2 * CH:(h2 + 1) * CH],
                                         start=False, stop=True)
                        nc.vector.tensor_copy(zs[:, h2 * CH:(h2 + 1) * CH], zf)
                    przU = ps_m.tile([128, CH], F32, tag="ps_m", name="cprz")
                    nc.tensor.matmul(przU[0:64, :], onesrow[0:1, :],
                                     zs[:, 0:CH], start=True, stop=True,
                                     tile_position=(0, 0))
                    nc.tensor.matmul(przU[64:128, :], onesrow[0:1, :],
                                     zs[:, CH:2 * CH], start=True, stop=True,
                                     tile_position=(0, 64))
                    rec = zsm2.tile([128, CH], F32, tag="rec", bufs=2,
                                    name="crec")
                    nc.vector.reciprocal_approx_fast(out=rec, in_=przU)
                    nc.vector.tensor_mul(at, at, rec)

            # ---- CA WO + residual(x1) + cvec + LN -> y1, y1T ----
            with nc.named_scope("ca_wo_ln"):
                cvec_t = bcast_row(d_cvec, "vec", "cvec")
                g2 = bcast_row(d_gbt["ca_g"], "gt", "g2")
                bt2 = bcast_row(d_gbt["ca_bt"], "gt", "bt2")
                wo = load_w8(d_w["ca_wo"])
                y1 = []
                for mt in range(NMT):
                    pre = resp.tile([128, D], F32, tag="persist",
                                    name=f"y1_{mt}")
                    for n in range(2):
                        ps = ps_m.tile([128, 512], F32, tag="ps_m",
                                       name="cwops")
                        for k in range(DT):
                            nc.tensor.matmul(
                                ps, aun[k][:, mt * 128:(mt + 1) * 128],
                                wo[k][:, n * 512:(n + 1) * 512],
                                start=(k == 0), stop=(k == DT - 1))
                        nc.vector.tensor_add(pre[:, n * 512:(n + 1) * 512],
                                             ps,
                                             x1[mt][:, n * 512:(n + 1) * 512])
                    nc.vector.tensor_add(pre, pre, cvec_t)
                    layer_norm(pre, g2, bt2, pre)
                    y1.append(pre)
                y1T = transposeT(y1, "y1T")

        # =================== FFN ==========================================
        with ExitStack() as ffn_ctx:
            hpool = ffn_ctx.enter_context(tc.tile_pool(name="hpool", bufs=32))
            w1pool = ffn_ctx.enter_context(tc.tile_pool(name="w1pool", bufs=8))
            with nc.named_scope("ffn1"):
                w1 = []
                for k in range(DT):
                    t = w1pool.tile([128, F], BF16, tag="w1", name=f"w1_{k}")
                    nc.sync.dma_start(out=t,
                                      in_=d_w1.ap()[k * 128:(k + 1) * 128, :])
                    w1.append(t)
                hT = []
                for m in range(NFT):
                    ps = ps_m.tile([128, CH], F32, tag="ps_m", name="f1ps")
                    for k in range(DT):
                        nc.tensor.matmul(ps, w1[k][:, m * 128:(m + 1) * 128],
                                         y1T[k], start=(k == 0),
                                         stop=(k == DT - 1))
                    h = hpool.tile([128, CH], BF16, tag="h", name=f"h{m}")
                    nc.scalar.activation(out=h, in_=ps, func=AF.Relu,
                                         bias=b1c[:, m:m + 1], scale=1.0)
                    hT.append(h)
            with nc.named_scope("ffn2"):
                b2v_t = bcast_row(d_b2v, "vec", "b2v")
                h2 = [resp.tile([128, D], F32, tag="persist", name=f"h2_{i}")
                      for i in range(NMT)]
                for n in range(2):
                    pss = [ps_s.tile([128, 2 * CH], F32, tag="ps_s",
                                     name=f"f2ps{n}_{i}") for i in range(2)]
                    for kb in range(4):
                        w2b = w1pool.tile([128, 8, 512], BF16, tag="w1",
                                          name=f"w2b{kb}")
                        nc.sync.dma_start(
                            out=w2b,
                            in_=bass.AP(tensor=d_w2.ap().tensor,
                                        offset=kb * 8 * 128 * D + n * 512,
                                        ap=[[D, 128], [128 * D, 8], [1, 512]]))
                        for ks in range(8):
                            k = kb * 8 + ks
                            for mt in range(NMT):
                                nc.tensor.matmul(
                                    pss[mt // 2][:, (mt % 2) * CH:
                                                 (mt % 2 + 1) * CH],
                                    hT[k][:, mt * 128:(mt + 1) * 128],
                                    w2b[:, ks, :],
                                    start=(k == 0), stop=(k == NFT - 1))
                    for mt in range(NMT):
                        nc.vector.tensor_add(
                            h2[mt][:, n * 512:(n + 1) * 512],
                            pss[mt // 2][:, (mt % 2) * CH:(mt % 2 + 1) * CH],
                            y1[mt][:, n * 512:(n + 1) * 512])
            with nc.named_scope("ln3_out"):
                g3 = bcast_row(d_gbt["f_g"], "gt", "g3")
                bt3 = bcast_row(d_gbt["f_bt"], "gt", "bt3")
                for mt in range(NMT):
                    nc.vector.tensor_add(h2[mt], h2[mt], b2v_t)
                    layer_norm(h2[mt], g3, bt3, h2[mt])
                    nc.sync.dma_start(out=d_out.ap()[mt * 128:(mt + 1) * 128, :],
                                      in_=h2[mt])

    nc.compile()
    return nc


def _bf(a):
    return np.ascontiguousarray(a, dtype=np.float32).astype(ml_dtypes.bfloat16)


def kernel(**inputs):
    global _CACHED
    if _CACHED is None:
        _CACHED = build()
    nc = _CACHED

    f = {k: np.asarray(v, dtype=np.float32) for k, v in inputs.items()}
    dec, enc = f["decoder_input"], f["encoder_output"]
    cvec = (f["ca_bv"] @ f["ca_wo"] + f["ca_bo"]).astype(np.float32)
    r1vec = (f["sa_bv"] @ f["sa_wo"] + f["sa_bo"]).astype(np.float32)

    shared = {n: _bf(f[n]) for n in
              ["ca_wq", "ca_wk", "ca_wv", "ca_wo", "f_w1", "f_w2"]}
    shared.update({n: f[n] for n in ["ca_bq", "ca_bk", "f_b1"]})
    shared["cvec"] = _bf(cvec)
    shared["b2v"] = _bf(f["f_b2"])
    for n in ["sa_g", "sa_bt", "ca_g", "ca_bt", "f_g", "f_bt"]:
        shared[n] = _bf(f[n])

    in_maps = []
    for c in range(8):
        b, j = c // 4, c % 4
        rows = slice(j * CH, (j + 1) * CH)
        hs = slice(j * 256, (j + 1) * 256)
        m = {
            "xT": _bf(dec[b].T),
            "eTq": _bf(enc[b, rows, :].T),
            "res1b": _bf(dec[b, rows, :] + r1vec),
            "sa_wq_h": _bf(f["sa_wq"][:, hs]),
            "sa_wk_h": _bf(f["sa_wk"][:, hs]),
            "sa_wv_h": _bf(f["sa_wv"][:, hs]),
            "sa_wo_h": _bf(f["sa_wo"][hs, :]),
            "sa_bq_h": np.ascontiguousarray(f["sa_bq"][hs], dtype=np.float32),
            "sa_bk_h": np.ascontiguousarray(f["sa_bk"][hs], dtype=np.float32),
        }
        m.update(shared)
        in_maps.append(m)

    global LAST_RES
    res = bass_utils.run_bass_kernel_spmd(nc, in_maps, core_ids=list(range(8)))
    LAST_RES = res
    out = np.empty((B, S, D), dtype=np.float32)
    for c in range(8):
        b, j = c // 4, c % 4
        out[b, j * CH:(j + 1) * CH, :] = res.results[c]["out"]
    return out


# revision 8
# speedup vs baseline: 1.0978x; 1.0978x over previous
"""Transformer decoder block (self-attn + cross-attn + FFN, post-LN) on 8
Trainium2 NeuronCores.

v3: head-sharded causal self-attention + token-sharded cross-attn/FFN.

8 cores = 2 batches x 4 ranks. Rank j of a batch group:
  - SA: computes heads [4j, 4j+4) for ALL 2048 tokens. K/V/Q projected
    locally from the full decoder input (no collective before attention).
    Causal structure is uniform across cores: per 512-query chunk qc only
    key tiles kt < 4(qc+1) are computed (62.5% of the full score work) and
    only the 4 diagonal tiles are masked. WO partials [2048, 1024] are
    staged in fp8e3 and summed across the 4 ranks with two column-split
    ReduceScatters; rank j receives its own 512-token slice.
  - CA: token-sharded as v2 — each rank projects K/V for its 512 encoder
    tokens, one fused AllGather per tensor (fp8e3), attention for its 512
    queries over all 2048 keys. The CA AllGathers are issued early and fly
    under the SA compute.
  - FFN + all residual/LN paths: token-sharded (512 tokens per rank).

v3 micro-optimizations vs v2:
  - CA score/AV matmuls read the fp8 AllGather buffers directly as the
    stationary operand (mixed fp8xbf16 matmul) — the fp8->bf16 DVE casts
    are gone.
  - K/V transport in fp8e3 (e3m4) instead of e4m3: halves the
    quantization error of the collective path.
  - softmax 1/Z via reciprocal_approx_fast (~5x faster than reciprocal).
  - Z-accumulation split between the DVE and the (otherwise idle) GpSimd
    engine: two partial accumulators, merged by the partition-sum matmul.
  - residual 1 (decoder input + bv@wo+bo) precomputed host-side and DMAd
    in [token, feature] layout directly (no PE transposes to rebuild it).
All matmuls bf16 (or fp8e3 stationary) with fp32 PSUM accumulation.
"""

from contextlib import ExitStack

import numpy as np
import ml_dtypes

import concourse.bass as bass
import concourse.bacc as bacc
import concourse.mybir as mybir
import concourse.tile as tile
from concourse import bass_utils
from concourse.masks import make_identity

BF16 = mybir.dt.bfloat16
F8E3 = mybir.dt.float8e3
F32 = mybir.dt.float32
AF = mybir.ActivationFunctionType
OP = mybir.AluOpType

B, S, D, H, F = 2, 2048, 1024, 16, 4096
DH = 64
EPS = 1e-5
CH = 512          # output tokens per core
DT = D // 128     # 8 feature tiles
NKT = S // 128    # 16 key tiles
NMT = CH // 128   # 4 token tiles per core (output)
NFT = F // 128    # 32 FFN hidden tiles
NST = S // 128    # 16 token tiles (full sequence)

_CACHED = None


def build():
    nc = bacc.Bacc("TRN2", target_bir_lowering=False, debug=False,
                   enable_asserts=False, num_devices=8)

    # ---- per-core DRAM I/O ----
    d_xT = nc.dram_tensor("xT", [D, S], BF16, kind="ExternalInput")
    d_eTq = nc.dram_tensor("eTq", [D, CH], BF16, kind="ExternalInput")
    d_res1b = nc.dram_tensor("res1b", [CH, D], BF16, kind="ExternalInput")
    d_wqh = nc.dram_tensor("sa_wq_h", [D, 256], BF16, kind="ExternalInput")
    d_wkh = nc.dram_tensor("sa_wk_h", [D, 256], BF16, kind="ExternalInput")
    d_wvh = nc.dram_tensor("sa_wv_h", [D, 256], BF16, kind="ExternalInput")
    d_woh = nc.dram_tensor("sa_wo_h", [256, D], BF16, kind="ExternalInput")
    d_bqh = nc.dram_tensor("sa_bq_h", [256], F32, kind="ExternalInput")
    d_bkh = nc.dram_tensor("sa_bk_h", [256], F32, kind="ExternalInput")
    cnames = ["ca_wq", "ca_wk", "ca_wv", "ca_wo"]
    d_w = {n: nc.dram_tensor(n, [D, D], BF16, kind="ExternalInput")
           for n in cnames}
    d_w1 = nc.dram_tensor("f_w1", [D, F], BF16, kind="ExternalInput")
    d_w2 = nc.dram_tensor("f_w2", [F, D], BF16, kind="ExternalInput")
    d_bq_ca = nc.dram_tensor("ca_bq", [D], F32, kind="ExternalInput")
    d_bk_ca = nc.dram_tensor("ca_bk", [D], F32, kind="ExternalInput")
    d_b1 = nc.dram_tensor("f_b1", [F], F32, kind="ExternalInput")
    d_cvec = nc.dram_tensor("cvec", [D], BF16, kind="ExternalInput")
    d_b2v = nc.dram_tensor("b2v", [D], BF16, kind="ExternalInput")
    d_gbt = {n: nc.dram_tensor(n, [D], BF16, kind="ExternalInput")
             for n in ["sa_g", "sa_bt", "ca_g", "ca_bt", "f_g", "f_bt"]}
    d_out = nc.dram_tensor("out", [CH, D], F32, kind="ExternalOutput")

    cc = {
        "kt_in": nc.dram_tensor("cc_ca_kt_in", [D, CH], F8E3, kind="Internal"),
        "kt_out": nc.dram_tensor("cc_ca_kt_out", [4 * D, CH], F8E3,
                                 kind="Internal"),
        "v_in": nc.dram_tensor("cc_ca_v_in", [CH, D], F8E3, kind="Internal"),
        "v_out": nc.dram_tensor("cc_ca_v_out", [S, D], F8E3, kind="Internal"),
        "rs_in": nc.dram_tensor("cc_rs_in", [S, D], F8E3, kind="Internal"),
        "rs_out": nc.dram_tensor("cc_rs_out", [CH, D], F8E3, kind="Internal"),
    }
    GROUPS = [[0, 1, 2, 3], [4, 5, 6, 7]]

    with tile.TileContext(nc) as tc, ExitStack() as ctx:
        const = ctx.enter_context(tc.tile_pool(name="const", bufs=1))
        wpool = ctx.enter_context(tc.tile_pool(name="wpool", bufs=8))
        qpool = ctx.enter_context(tc.tile_pool(name="qpool", bufs=16))
        resp = ctx.enter_context(tc.tile_pool(name="resp", bufs=8))
        scrp = ctx.enter_context(tc.tile_pool(name="scrp", bufs=2))
        ps_s = ctx.enter_context(tc.tile_pool(name="ps_s", bufs=2, space="PSUM"))
        ps_av = ctx.enter_context(tc.tile_pool(name="ps_av", bufs=2, space="PSUM"))
        ps_m = ctx.enter_context(tc.tile_pool(name="ps_m", bufs=3, space="PSUM"))

        ident = const.tile([128, 128], F32, tag="ident")
        make_identity(nc, ident)
        identb = const.tile([128, 128], BF16, tag="identb")
        nc.vector.tensor_copy(identb, ident)
        onescol = const.tile([128, 1], BF16, tag="onescol")
        nc.vector.memset(onescol, 1.0)
        onesrow = const.tile([1, 64], F32, tag="onesrow")
        nc.vector.memset(onesrow, 1.0)
        epst = const.tile([128, 1], F32, tag="epst")
        nc.vector.memset(epst, EPS)
        zerot = const.tile([128, 1], F32, tag="zerot")
        nc.vector.memset(zerot, 0.0)

        def bias_cols(dram, ntiles, name):
            t = const.tile([128, ntiles], F32, tag=name, name=name)
            src = bass.AP(tensor=dram.ap().tensor, offset=0,
                          ap=[[1, 128], [128, ntiles]])
            nc.sync.dma_start(out=t, in_=src)
            return t

        def bcast_row(dram, tag, name):
            t = const.tile([128, D], BF16, tag=tag, bufs=2, name=name)
            src = bass.AP(tensor=dram.ap().tensor, offset=0, ap=[[0, 128], [1, D]])
            nc.sync.dma_start(out=t, in_=src)
            return t

        bq_h = bias_cols(d_bqh, 2, "bqh")
        bk_h = bias_cols(d_bkh, 2, "bkh")
        bq_ca = bias_cols(d_bq_ca, DT, "bqca")
        bk_ca = bias_cols(d_bk_ca, DT, "bkca")
        b1c = bias_cols(d_b1, NFT, "b1c")

        def layer_norm(src, g_t, bt_t, out):
            """[128, D] f32 LN along free dim; out may alias src."""
            stats = scrp.tile([128, 2, 6], F32, tag="lnstat", name="lnstat")
            for s in range(2):
                nc.vector.bn_stats(out=stats[:, s, :],
                                   in_=src[:, s * 512:(s + 1) * 512])
            mv = scrp.tile([128, 2], F32, tag="lnmv", name="lnmv")
            nc.vector.bn_aggr(out=mv, in_=stats)
            rstd = scrp.tile([128, 1], F32, tag="lnrstd", name="lnrstd")
            nc.scalar.activation(out=rstd, in_=mv[:, 1:2], func=AF.Sqrt,
                                 bias=epst, scale=1.0)
            nc.vector.reciprocal(out=rstd, in_=rstd)
            cent = scrp.tile([128, D], F32, tag="scr", name="cent")
            nc.vector.scalar_tensor_tensor(out=cent, in0=src, scalar=mv[:, 0:1],
                                           in1=g_t, op0=OP.subtract, op1=OP.mult)
            nc.vector.scalar_tensor_tensor(out=out, in0=cent, scalar=rstd,
                                           in1=bt_t, op0=OP.mult, op1=OP.add)

        def load_w8(wd, ncols=D):
            ws = []
            for k in range(DT):
                t = wpool.tile([128, ncols], BF16, tag="w", name=f"w_{k}")
                nc.sync.dma_start(out=t, in_=wd.ap()[k * 128:(k + 1) * 128, :])
                ws.append(t)
            return ws

        # =================== CA K/V local + AllGathers (first) ============
        with nc.named_scope("ca_kvlocal"):
            eq = []
            for k in range(DT):
                t = qpool.tile([128, CH], BF16, tag="qt", name=f"eq{k}")
                nc.sync.dma_start(out=t, in_=d_eTq.ap()[k * 128:(k + 1) * 128, :])
                eq.append(t)
            wk = load_w8(d_w["ca_wk"])
            for m in range(DT):
                ps = ps_m.tile([128, CH], F32, tag="ps_m", name="lkps")
                for k in range(DT):
                    nc.tensor.matmul(ps, wk[k][:, m * 128:(m + 1) * 128],
                                     eq[k], start=(k == 0), stop=(k == DT - 1))
                st = scrp.tile([128, CH], F8E3, tag="stage", bufs=4, name="ktst")
                nc.scalar.activation(out=st, in_=ps, func=AF.Identity,
                                     bias=bk_ca[:, m:m + 1], scale=1.0)
                nc.sync.dma_start(
                    out=cc["kt_in"].ap()[m * 128:(m + 1) * 128, :], in_=st)
            nc.gpsimd.collective_compute(
                "AllGather", mybir.AluOpType.bypass,
                ins=[cc["kt_in"].ap()], outs=[cc["kt_out"].ap()],
                replica_groups=GROUPS)
            wv = load_w8(d_w["ca_wv"])
            for tt in range(NMT):
                for n in range(2):
                    ps = ps_m.tile([128, CH], F32, tag="ps_m", name="lvps")
                    for k in range(DT):
                        nc.tensor.matmul(
                            ps, eq[k][:, tt * 128:(tt + 1) * 128],
                            wv[k][:, n * 512:(n + 1) * 512],
                            start=(k == 0), stop=(k == DT - 1))
                    st = scrp.tile([128, CH], F8E3, tag="stage", bufs=4,
                                   name="vst")
                    nc.scalar.activation(out=st, in_=ps, func=AF.Copy)
                    nc.sync.dma_start(
                        out=cc["v_in"].ap()[tt * 128:(tt + 1) * 128,
                                            n * 512:(n + 1) * 512],
                        in_=st)
            nc.gpsimd.collective_compute(
                "AllGather", mybir.AluOpType.bypass,
                ins=[cc["v_in"].ap()], outs=[cc["v_out"].ap()],
                replica_groups=GROUPS)

        # =================== SA local projections =========================
        with ExitStack() as sa_ctx:
            big = sa_ctx.enter_context(tc.tile_pool(name="big", bufs=2))
            maskp = sa_ctx.enter_context(tc.tile_pool(name="maskp", bufs=1))
            ppool = sa_ctx.enter_context(tc.tile_pool(name="ppool", bufs=3))
            zpool = sa_ctx.enter_context(tc.tile_pool(name="zpool", bufs=2))
            zsm = sa_ctx.enter_context(tc.tile_pool(name="zsm", bufs=2))

            # K^T and Q^T: [128 (2 heads x 64 dh), 2048 tokens] per pair
            KT = [big.tile([128, S], BF16, tag="KT", bufs=2, name=f"KT{g}")
                  for g in range(2)]
            QT = [big.tile([128, S], BF16, tag="QT", bufs=2, name=f"QT{g}")
                  for g in range(2)]
            # V: [128 tokens per tile, 16 tiles, 256 dh]  (bias folded)
            vt = big.tile([128, NST, 256], F8E3, tag="vt", bufs=1, name="vt")

            with ExitStack() as proj_ctx, nc.named_scope("sa_proj"):
                projp = proj_ctx.enter_context(
                    tc.tile_pool(name="projp", bufs=8))
                xt = []
                for k in range(DT):
                    t = projp.tile([128, S], BF16, tag="xt", bufs=8,
                                   name=f"xt{k}")
                    nc.sync.dma_start(out=t,
                                      in_=d_xT.ap()[k * 128:(k + 1) * 128, :])
                    xt.append(t)

                def load_wh(wd, tag):
                    ws = []
                    for k in range(DT):
                        t = projp.tile([128, 256], BF16, tag=tag, bufs=8,
                                       name=f"{tag}{k}")
                        nc.sync.dma_start(
                            out=t, in_=wd.ap()[k * 128:(k + 1) * 128, :])
                        ws.append(t)
                    return ws

                wqh = load_wh(d_wqh, "wqh")
                wkh = load_wh(d_wkh, "wkh")
                wvh = load_wh(d_wvh, "wvh")

                for g in range(2):
                    for tc4 in range(4):
                        ps = ps_m.tile([128, CH], F32, tag="ps_m", name="kps")
                        for k in range(DT):
                            nc.tensor.matmul(
                                ps, wkh[k][:, g * 128:(g + 1) * 128],
                                xt[k][:, tc4 * 512:(tc4 + 1) * 512],
                                start=(k == 0), stop=(k == DT - 1))
                        nc.scalar.activation(
                            out=KT[g][:, tc4 * 512:(tc4 + 1) * 512], in_=ps,
                            func=AF.Identity, bias=bk_h[:, g:g + 1], scale=1.0)
                for g in range(2):
                    for tc4 in range(4):
                        ps = ps_m.tile([128, CH], F32, tag="ps_m", name="qps")
                        for k in range(DT):
                            nc.tensor.matmul(
                                ps, wqh[k][:, g * 128:(g + 1) * 128],
                                xt[k][:, tc4 * 512:(tc4 + 1) * 512],
                                start=(k == 0), stop=(k == DT - 1))
                        nc.scalar.activation(
                            out=QT[g][:, tc4 * 512:(tc4 + 1) * 512], in_=ps,
                            func=AF.Identity, bias=bq_h[:, g:g + 1], scale=1.0)
                for tt in range(NST):
                    ps = ps_m.tile([128, 256], F32, tag="ps_m", name="vps")
                    for k in range(DT):
                        nc.tensor.matmul(
                            ps, xt[k][:, tt * 128:(tt + 1) * 128], wvh[k],
                            start=(k == 0), stop=(k == DT - 1))
                    nc.scalar.activation(out=vt[:, tt, :], in_=ps,
                                         func=AF.Copy)

            # ---- causal mask for the 4 diagonal tiles (same on all cores):
            # amask[k, dkt, q] = -240 if (128*dkt + k > q) else 0
            qmk = scrp.tile([128, CH], F32, tag="qmk", bufs=1, name="qmk")
            nc.gpsimd.iota(qmk, pattern=[[1, CH]], base=0,
                           channel_multiplier=-1,
                           allow_small_or_imprecise_dtypes=True)
            amask = maskp.tile([128, 4, CH], BF16, tag="mask", name="amask")
            for dkt in range(4):
                nc.vector.tensor_scalar(out=amask[:, dkt, :], in0=qmk,
                                        scalar1=float(128 * dkt),
                                        scalar2=-240.0, op0=OP.is_lt,
                                        op1=OP.mult)

            # =================== SA attention pairs =======================
            # qc-outer: after both pair-groups of a 512-token chunk finish,
            # its WO partial is computed and chunk-ReduceScatter qc fires --
            # RS 0..2 fly under the remaining pairs, only RS 3 is exposed.
            attnT = [big.tile([128, S], BF16, tag="atn", bufs=2,
                              name=f"atn{g}") for g in range(2)]
            woh = [big.tile([128, D], BF16, tag="woh", bufs=2,
                            name=f"woh{g}") for g in range(2)]
            for g in range(2):
                nc.sync.dma_start(
                    out=woh[g], in_=d_woh.ap()[g * 128:(g + 1) * 128, :])
            GPN = {0: 0, 1: 1, 2: 1, 3: 1}   # trailing kt2 iters on GpSimd
            for qc in range(4):
                for g in range(2):
                    with nc.named_scope(f"sa_g{g}q{qc}"):
                        nkt = 4 * qc + 4
                        nkt2 = nkt // 2
                        ngp = GPN[qc]
                        ndv = nkt2 - ngp
                        qa = QT[g][0:64, qc * 512:(qc + 1) * 512]
                        qb = QT[g][64:128, qc * 512:(qc + 1) * 512]
                        pav = ps_av.tile([128, CH], F32, tag="ps_av", bufs=1,
                                         name="pav")
                        zacc_d = zpool.tile([128, 2 * CH], BF16, tag="zd",
                                            bufs=2, name="zacc_d")
                        zacc_g = zpool.tile([128, 2 * CH], BF16, tag="zg",
                                            bufs=2, name="zacc_g")
                        for kt2 in range(nkt2):
                            pt2 = ppool.tile([128, 2, 2 * CH], BF16, tag="pt",
                                             name="pt")
                            for sub in range(2):
                                kt = 2 * kt2 + sub
                                diag = kt >= 4 * qc
                                pss = ps_s.tile([128, 2 * CH], F32, tag="ps_s",
                                                name="pss")
                                ksl = KT[g][:, kt * 128:(kt + 1) * 128]
                                nc.tensor.matmul(pss[:, 0:CH], ksl[0:64, :],
                                                 qa, start=True,
                                                 stop=not diag)
                                nc.tensor.matmul(pss[:, CH:2 * CH],
                                                 ksl[64:128, :], qb,
                                                 start=True, stop=True)
                                if diag:
                                    dkt = kt - 4 * qc
                                    nc.tensor.matmul(pss[:, 0:CH], identb,
                                                     amask[:, dkt, :],
                                                     start=False, stop=True,
                                                     skip_group_check=True)
                                nc.scalar.activation(
                                    out=pt2[:, sub, :], in_=pss, func=AF.Exp,
                                    bias=zerot, scale=1.0 / np.sqrt(DH))
                                if diag:
                                    dkt = kt - 4 * qc
                                    nc.vector.scalar_tensor_tensor(
                                        out=pt2[:, sub, CH:2 * CH],
                                        in0=amask[:, dkt, :],
                                        scalar=-1.0, op0=OP.is_ge,
                                        in1=pt2[:, sub, CH:2 * CH],
                                        op1=OP.mult)
                            if kt2 < ndv:
                                eng, zt, first = nc.vector, zacc_d, kt2 == 0
                            else:
                                eng, zt, first = nc.gpsimd, zacc_g, kt2 == ndv
                            if first:
                                eng.tensor_copy(zt, pt2[:, 0, :])
                            else:
                                eng.tensor_add(zt, zt, pt2[:, 0, :])
                            eng.tensor_add(zt, zt, pt2[:, 1, :])
                            for sub in range(2):
                                kt = 2 * kt2 + sub
                                vsl = vt[:, kt, g * 128:(g + 1) * 128]
                                nc.tensor.matmul(pav[0:64, :], vsl[:, 0:64],
                                                 pt2[:, sub, 0:CH],
                                                 start=(kt == 0),
                                                 stop=(kt == nkt - 1))
                                nc.tensor.matmul(pav[64:128, :],
                                                 vsl[:, 64:128],
                                                 pt2[:, sub, CH:2 * CH],
                                                 start=(kt == 0),
                                                 stop=(kt == nkt - 1))
                        at = attnT[g][:, qc * 512:(qc + 1) * 512]
                        nc.vector.tensor_copy(at, pav)
                        zs = zsm.tile([1, 2 * CH], F32, tag="zs", bufs=2,
                                      name="zs")
                        for h2 in range(2):
                            zf = ps_m.tile([1, CH], F32, tag="ps_m", name="zf")
                            nc.tensor.matmul(zf, onescol,
                                             zacc_d[:, h2 * CH:(h2 + 1) * CH],
                                             start=True, stop=(ngp == 0))
                            if ngp:
                                nc.tensor.matmul(
                                    zf, onescol,
                                    zacc_g[:, h2 * CH:(h2 + 1) * CH],
                                    start=False, stop=True)
                            nc.vector.tensor_copy(zs[:, h2 * CH:(h2 + 1) * CH],
                                                  zf)
                        przU = ps_m.tile([128, CH], F32, tag="ps_m",
                                         name="przU")
                        nc.tensor.matmul(przU[0:64, :], onesrow[0:1, :],
                                         zs[:, 0:CH], start=True, stop=True,
                                         tile_position=(0, 0))
                        nc.tensor.matmul(przU[64:128, :], onesrow[0:1, :],
                                         zs[:, CH:2 * CH], start=True,
                                         stop=True, tile_position=(0, 64))
                        rec = zsm.tile([128, CH], F32, tag="rec", bufs=2,
                                       name="rec")
                        nc.vector.reciprocal_approx_fast(out=rec, in_=przU)
                        nc.vector.tensor_mul(at, at, rec)
                # ---- WO partial for this 512-token chunk + chunk RS ----
                with nc.named_scope(f"sa_wo{qc}"):
                    for mt in range(4 * qc, 4 * qc + 4):
                        st = scrp.tile([128, D], F8E3, tag="stage2", bufs=4,
                                       name="rsst")
                        for n in range(2):
                            ps = ps_m.tile([128, CH], F32, tag="ps_m",
                                           name="wops")
                            for g in range(2):
                                nc.tensor.matmul(
                                    ps, attnT[g][:, mt * 128:(mt + 1) * 128],
                                    woh[g][:, n * 512:(n + 1) * 512],
                                    start=(g == 0), stop=(g == 1))
                            if (mt + n) % 2 == 0:
                                nc.scalar.activation(
                                    out=st[:, n * 512:(n + 1) * 512], in_=ps,
                                    func=AF.Copy)
                            else:
                                nc.vector.tensor_copy(
                                    st[:, n * 512:(n + 1) * 512], ps)
                        nc.sync.dma_start(
                            out=cc["rs_in"].ap()[mt * 128:(mt + 1) * 128, :],
                            in_=st)
                    nc.gpsimd.collective_compute(
                        "ReduceScatter", mybir.AluOpType.add,
                        ins=[cc["rs_in"].ap()[qc * 512:(qc + 1) * 512, :]],
                        outs=[cc["rs_out"].ap()[qc * 128:(qc + 1) * 128, :]],
                        replica_groups=GROUPS)

        # =================== post-RS: x1 = LN(RS + res1b) =================
        def transposeT(x_tiles, out_tag):
            """4 [128, D] f32 token-major -> 8 [128, CH] bf16 feature-major."""
            outs = [qpool.tile([128, CH], BF16, tag="qt",
                               name=f"{out_tag}{i}") for i in range(DT)]
            for mt in range(NMT):
                for ft in range(DT):
                    pst = ps_m.tile([128, 128], F32, tag="ps_m", name="tps")
                    nc.tensor.transpose(
                        pst, x_tiles[mt][:, ft * 128:(ft + 1) * 128], ident)
                    nc.vector.tensor_copy(
                        outs[ft][:, mt * 128:(mt + 1) * 128], pst)
            return outs

        with nc.named_scope("sa_ln"):
            g1 = bcast_row(d_gbt["sa_g"], "gt", "g1")
            bt1 = bcast_row(d_gbt["sa_bt"], "gt", "bt1")
            x1 = []
            for mt in range(NMT):
                rst = scrp.tile([128, D], F8E3, tag="rst", bufs=2, name="rst")
                nc.sync.dma_start(
                    out=rst, in_=cc["rs_out"].ap()[mt * 128:(mt + 1) * 128, :])
                r1t = scrp.tile([128, D], BF16, tag="r1t", bufs=2, name="r1t")
                nc.sync.dma_start(
                    out=r1t, in_=d_res1b.ap()[mt * 128:(mt + 1) * 128, :])
                pre = resp.tile([128, D], F32, tag="persist", name=f"x1_{mt}")
                nc.vector.tensor_add(pre, rst, r1t)
                layer_norm(pre, g1, bt1, pre)
                x1.append(pre)
            x1T = transposeT(x1, "x1T")

        # =================== CA attention =================================
        with ExitStack() as ca_ctx:
            kvp = ca_ctx.enter_context(tc.tile_pool(name="kvp", bufs=3))
            vpp = ca_ctx.enter_context(tc.tile_pool(name="vpp", bufs=1))
            ppool2 = ca_ctx.enter_context(tc.tile_pool(name="ppool2", bufs=4))
            zpool2 = ca_ctx.enter_context(tc.tile_pool(name="zpool2", bufs=2))
            attp = ca_ctx.enter_context(tc.tile_pool(name="attp", bufs=8))
            zsm2 = ca_ctx.enter_context(tc.tile_pool(name="zsm2", bufs=2))

            with nc.named_scope("ca_q"):
                wqc = load_w8(d_w["ca_wq"])
                QTca = []
                for m in range(DT):
                    ps = ps_m.tile([128, CH], F32, tag="ps_m", name="cqps")
                    for k in range(DT):
                        nc.tensor.matmul(ps, wqc[k][:, m * 128:(m + 1) * 128],
                                         x1T[k], start=(k == 0),
                                         stop=(k == DT - 1))
                    o = qpool.tile([128, CH], BF16, tag="qt", name=f"qca{m}")
                    nc.scalar.activation(out=o, in_=ps, func=AF.Identity,
                                         bias=bq_ca[:, m:m + 1], scale=1.0)
                    QTca.append(o)

            aun = []
            vcur = None
            for hp in range(H // 2):
                with nc.named_scope(f"ca_pair{hp}"):
                    # K^T for this pair straight from the fp8e3 AG buffer
                    ktp = kvp.tile([128, 4, CH], F8E3, tag="ktp", name="ktp")
                    nc.sync.dma_start(
                        out=ktp,
                        in_=bass.AP(tensor=cc["kt_out"].ap().tensor,
                                    offset=128 * hp * CH,
                                    ap=[[CH, 128], [D * CH, 4], [1, CH]]))
                    ktb = ktp.rearrange("p a q -> p (a q)")
                    if hp % 2 == 0:
                        vte = vpp.tile([128, NKT, 256], F8E3, tag="vpp",
                                       bufs=1, name="vpp")
                        nc.sync.dma_start(
                            out=vte,
                            in_=bass.AP(tensor=cc["v_out"].ap().tensor,
                                        offset=(hp // 2) * 256,
                                        ap=[[D, 128], [128 * D, NKT],
                                            [1, 256]]))
                        vcur = vte
                    voff = (hp % 2) * 128

                    qa = QTca[hp][0:64, :]
                    qb = QTca[hp][64:128, :]
                    pav = ps_av.tile([128, CH], F32, tag="ps_av", bufs=1,
                                     name="cpav")
                    zacc_d = zpool2.tile([128, 2 * CH], BF16, tag="zd",
                                         bufs=2, name="czd")
                    zacc_g = zpool2.tile([128, 2 * CH], BF16, tag="zg",
                                         bufs=2, name="czg")
                    NDV = 6   # kt2 0..5 on DVE, 6..7 on GpSimd
                    for kt2 in range(NKT // 2):
                        pt2 = ppool2.tile([128, 2, 2 * CH], BF16, tag="pt",
                                          name="cpt")
                        for sub in range(2):
                            kt = 2 * kt2 + sub
                            pss = ps_s.tile([128, 2 * CH], F32, tag="ps_s",
                                            name="cpss")
                            ksl = ktb[:, kt * 128:(kt + 1) * 128]
                            nc.tensor.matmul(pss[:, 0:CH], ksl[0:64, :], qa,
                                             start=True, stop=True)
                            nc.tensor.matmul(pss[:, CH:2 * CH],
                                             ksl[64:128, :], qb,
                                             start=True, stop=True)
                            nc.scalar.activation(
                                out=pt2[:, sub, :], in_=pss, func=AF.Exp,
                                bias=zerot, scale=1.0 / np.sqrt(DH))
                        if kt2 < NDV:
                            eng, zt, first = nc.vector, zacc_d, kt2 == 0
                        else:
                            eng, zt, first = nc.gpsimd, zacc_g, kt2 == NDV
                        if first:
                            eng.tensor_copy(zt, pt2[:, 0, :])
                        else:
                            eng.tensor_add(zt, zt, pt2[:, 0, :])
                        eng.tensor_add(zt, zt, pt2[:, 1, :])
                        for sub in range(2):
                            kt = 2 * kt2 + sub
                            vsl = vcur[:, kt, voff:voff + 128]
                            nc.tensor.matmul(pav[0:64, :], vsl[:, 0:64],
                                             pt2[:, sub, 0:CH],
                                             start=(kt == 0),
                                             stop=(kt == NKT - 1))
                            nc.tensor.matmul(pav[64:128, :], vsl[:, 64:128],
                                             pt2[:, sub, CH:2 * CH],
                                             start=(kt == 0),
                                             stop=(kt == NKT - 1))
                    at = attp.tile([128, CH], BF16, tag="aun", name=f"aun{hp}")
                    nc.vector.tensor_copy(at, pav)
                    aun.append(at)
                    zs = zsm2.tile([1, 2 * CH], F32, tag="zs", bufs=2,
                                   name="czs")
                    for h2 in range(2):
                        zf = ps_m.tile([1, CH], F32, tag="ps_m", name="czf")
                        nc.tensor.matmul(zf, onescol,
                                         zacc_d[:, h2 * CH:(h2 + 1) * CH],
                                         start=True, stop=False)
                        nc.tensor.matmul(zf, onescol,
                                         zacc_g[:, h2 * CH:(h2 + 1) * CH],
                                         start=False, stop=True)
                        nc.vector.tensor_copy(zs[:, h2 * CH:(h2 + 1) * CH], zf)
                    przU = ps_m.tile([128, CH], F32, tag="ps_m", name="cprz")
                    nc.tensor.matmul(przU[0:64, :], onesrow[0:1, :],
                                     zs[:, 0:CH], start=True, stop=True,
                                     tile_position=(0, 0))
                    nc.tensor.matmul(przU[64:128, :], onesrow[0:1, :],
                                     zs[:, CH:2 * CH], start=True, stop=True,
                                     tile_position=(0, 64))
                    rec = zsm2.tile([128, CH], F32, tag="rec", bufs=2,
                                    name="crec")
                    nc.vector.reciprocal_approx_fast(out=rec, in_=przU)
                    nc.vector.tensor_mul(at, at, rec)

            # ---- CA WO + residual(x1) + cvec + LN -> y1, y1T ----
            with nc.named_scope("ca_wo_ln"):
                cvec_t = bcast_row(d_cvec, "vec", "cvec")
                g2 = bcast_row(d_gbt["ca_g"], "gt", "g2")
                bt2 = bcast_row(d_gbt["ca_bt"], "gt", "bt2")
                wo = load_w8(d_w["ca_wo"])
                y1 = []
                for mt in range(NMT):
                    pre = resp.tile([128, D], F32, tag="persist",
                                    name=f"y1_{mt}")
                    for n in range(2):
                        ps = ps_m.tile([128, 512], F32, tag="ps_m",
                                       name="cwops")
                        for k in range(DT):
                            nc.tensor.matmul(
                                ps, aun[k][:, mt * 128:(mt + 1) * 128],
                                wo[k][:, n * 512:(n + 1) * 512],
                                start=(k == 0), stop=(k == DT - 1))
                        nc.vector.tensor_add(pre[:, n * 512:(n + 1) * 512],
                                             ps,
                                             x1[mt][:, n * 512:(n + 1) * 512])
                    nc.vector.tensor_add(pre, pre, cvec_t)
                    layer_norm(pre, g2, bt2, pre)
                    y1.append(pre)
                y1T = transposeT(y1, "y1T")

        # =================== FFN ==========================================
        with ExitStack() as ffn_ctx:
            hpool = ffn_ctx.enter_context(tc.tile_pool(name="hpool", bufs=32))
            w1pool = ffn_ctx.enter_context(tc.tile_pool(name="w1pool", bufs=8))
            with nc.named_scope("ffn1"):
                w1 = []
                for k in range(DT):
                    t = w1pool.tile([128, F], BF16, tag="w1", name=f"w1_{k}")
                    nc.sync.dma_start(out=t,
                                      in_=d_w1.ap()[k * 128:(k + 1) * 128, :])
                    w1.append(t)
                hT = []
                for m in range(NFT):
                    ps = ps_m.tile([128, CH], F32, tag="ps_m", name="f1ps")
                    for k in range(DT):
                        nc.tensor.matmul(ps, w1[k][:, m * 128:(m + 1) * 128],
                                         y1T[k], start=(k == 0),
                                         stop=(k == DT - 1))
                    h = hpool.tile([128, CH], BF16, tag="h", name=f"h{m}")
                    nc.scalar.activation(out=h, in_=ps, func=AF.Relu,
                                         bias=b1c[:, m:m + 1], scale=1.0)
                    hT.append(h)
            with nc.named_scope("ffn2"):
                b2v_t = bcast_row(d_b2v, "vec", "b2v")
                h2 = [resp.tile([128, D], F32, tag="persist", name=f"h2_{i}")
                      for i in range(NMT)]
                for n in range(2):
                    pss = [ps_s.tile([128, 2 * CH], F32, tag="ps_s",
                                     name=f"f2ps{n}_{i}") for i in range(2)]
                    for kb in range(4):
                        w2b = w1pool.tile([128, 8, 512], BF16, tag="w1",
                                          name=f"w2b{kb}")
                        nc.sync.dma_start(
                            out=w2b,
                            in_=bass.AP(tensor=d_w2.ap().tensor,
                                        offset=kb * 8 * 128 * D + n * 512,
                                        ap=[[D, 128], [128 * D, 8], [1, 512]]))
                        for ks in range(8):
                            k = kb * 8 + ks
                            for mt in range(NMT):
                                nc.tensor.matmul(
                                    pss[mt // 2][:, (mt % 2) * CH:
                                                 (mt % 2 + 1) * CH],
                                    hT[k][:, mt * 128:(mt + 1) * 128],
                                    w2b[:, ks, :],
                                    start=(k == 0), stop=(k == NFT - 1))
                    for mt in range(NMT):
                        nc.vector.tensor_add(
                            h2[mt][:, n * 512:(n + 1) * 512],
                            pss[mt // 2][:, (mt % 2) * CH:(mt % 2 + 1) * CH],
                            y1[mt][:, n * 512:(n + 1) * 512])
            with nc.named_scope("ln3_out"):
                g3 = bcast_row(d_gbt["f_g"], "gt", "g3")
                bt3 = bcast_row(d_gbt["f_bt"], "gt", "bt3")
                for mt in range(NMT):
                    nc.vector.tensor_add(h2[mt], h2[mt], b2v_t)
                    layer_norm(h2[mt], g3, bt3, h2[mt])
                    nc.sync.dma_start(out=d_out.ap()[mt * 128:(mt + 1) * 128, :],
                                      in_=h2[mt])

    nc.compile()
    return nc


def _bf(a):
    return np.ascontiguousarray(a, dtype=np.float32).astype(ml_dtypes.bfloat16)


def kernel(**inputs):
    global _CACHED
    if _CACHED is None:
        _CACHED = build()
    nc = _CACHED

    f = {k: np.asarray(v, dtype=np.float32) for k, v in inputs.items()}
    dec, enc = f["decoder_input"], f["encoder_output"]
    cvec = (f["ca_bv"] @ f["ca_wo"] + f["ca_bo"]).astype(np.float32)
    r1vec = (f["sa_bv"] @ f["sa_wo"] + f["sa_bo"]).astype(np.float32)

    shared = {n: _bf(f[n]) for n in
              ["ca_wq", "ca_wk", "ca_wv", "ca_wo", "f_w1", "f_w2"]}
    shared.update({n: f[n] for n in ["ca_bq", "ca_bk", "f_b1"]})
    shared["cvec"] = _bf(cvec)
    shared["b2v"] = _bf(f["f_b2"])
    for n in ["sa_g", "sa_bt", "ca_g", "ca_bt", "f_g", "f_bt"]:
        shared[n] = _bf(f[n])

    in_maps = []
    rows_of = {}
    for c in range(8):
        b, j = c // 4, c % 4
        rows = np.concatenate([np.arange(512 * qc + 128 * j,
                                         512 * qc + 128 * j + 128)
                               for qc in range(4)])
        rows_of[c] = rows
        hs = slice(j * 256, (j + 1) * 256)
        m = {
            "xT": _bf(dec[b].T),
            "eTq": _bf(enc[b, rows, :].T),
            "res1b": _bf(dec[b, rows, :] + r1vec),
            "sa_wq_h": _bf(f["sa_wq"][:, hs]),
            "sa_wk_h": _bf(f["sa_wk"][:, hs]),
            "sa_wv_h": _bf(f["sa_wv"][:, hs]),
            "sa_wo_h": _bf(f["sa_wo"][hs, :]),
            "sa_bq_h": np.ascontiguousarray(f["sa_bq"][hs], dtype=np.float32),
            "sa_bk_h": np.ascontiguousarray(f["sa_bk"][hs], dtype=np.float32),
        }
        m.update(shared)
        in_maps.append(m)

    global LAST_RES
    res = bass_utils.run_bass_kernel_spmd(nc, in_maps, core_ids=list(range(8)))
    LAST_RES = res
    out = np.empty((B, S, D), dtype=np.float32)
    for c in range(8):
        b = c // 4
        out[b, rows_of[c], :] = res.results[c]["out"]
    return out


# revision 9
# speedup vs baseline: 1.1026x; 1.0044x over previous
"""Transformer decoder block (self-attn + cross-attn + FFN, post-LN) on 8
Trainium2 NeuronCores.

v3: head-sharded causal self-attention + token-sharded cross-attn/FFN.

8 cores = 2 batches x 4 ranks. Rank j of a batch group:
  - SA: computes heads [4j, 4j+4) for ALL 2048 tokens. K/V/Q projected
    locally from the full decoder input (no collective before attention).
    Causal structure is uniform across cores: per 512-query chunk qc only
    key tiles kt < 4(qc+1) are computed (62.5% of the full score work) and
    only the 4 diagonal tiles are masked. WO partials [2048, 1024] are
    staged in fp8e3 and summed across the 4 ranks with two column-split
    ReduceScatters; rank j receives its own 512-token slice.
  - CA: token-sharded as v2 — each rank projects K/V for its 512 encoder
    tokens, one fused AllGather per tensor (fp8e3), attention for its 512
    queries over all 2048 keys. The CA AllGathers are issued early and fly
    under the SA compute.
  - FFN + all residual/LN paths: token-sharded (512 tokens per rank).

v3 micro-optimizations vs v2:
  - CA score/AV matmuls read the fp8 AllGather buffers directly as the
    stationary operand (mixed fp8xbf16 matmul) — the fp8->bf16 DVE casts
    are gone.
  - K/V transport in fp8e3 (e3m4) instead of e4m3: halves the
    quantization error of the collective path.
  - softmax 1/Z via reciprocal_approx_fast (~5x faster than reciprocal).
  - Z-accumulation split between the DVE and the (otherwise idle) GpSimd
    engine: two partial accumulators, merged by the partition-sum matmul.
  - residual 1 (decoder input + bv@wo+bo) precomputed host-side and DMAd
    in [token, feature] layout directly (no PE transposes to rebuild it).
All matmuls bf16 (or fp8e3 stationary) with fp32 PSUM accumulation.
"""

from contextlib import ExitStack

import numpy as np
import ml_dtypes

import concourse.bass as bass
import concourse.bacc as bacc
import concourse.mybir as mybir
import concourse.tile as tile
from concourse import bass_utils
from concourse.masks import make_identity

BF16 = mybir.dt.bfloat16
F8E3 = mybir.dt.float8e3
F32 = mybir.dt.float32
AF = mybir.ActivationFunctionType
OP = mybir.AluOpType

B, S, D, H, F = 2, 2048, 1024, 16, 4096
DH = 64
EPS = 1e-5
CH = 512          # output tokens per core
DT = D // 128     # 8 feature tiles
NKT = S // 128    # 16 key tiles
NMT = CH // 128   # 4 token tiles per core (output)
NFT = F // 128    # 32 FFN hidden tiles
NST = S // 128    # 16 token tiles (full sequence)

_CACHED = None


def build():
    nc = bacc.Bacc("TRN2", target_bir_lowering=False, debug=False,
                   enable_asserts=False, num_devices=8)

    # ---- per-core DRAM I/O ----
    d_xT = nc.dram_tensor("xT", [D, S], BF16, kind="ExternalInput")
    d_eTq = nc.dram_tensor("eTq", [D, CH], BF16, kind="ExternalInput")
    d_res1b = nc.dram_tensor("res1b", [CH, D], BF16, kind="ExternalInput")
    d_wqh = nc.dram_tensor("sa_wq_h", [D, 256], BF16, kind="ExternalInput")
    d_wkh = nc.dram_tensor("sa_wk_h", [D, 256], BF16, kind="ExternalInput")
    d_wvh = nc.dram_tensor("sa_wv_h", [D, 256], BF16, kind="ExternalInput")
    d_woh = nc.dram_tensor("sa_wo_h", [256, D], BF16, kind="ExternalInput")
    d_bqh = nc.dram_tensor("sa_bq_h", [256], F32, kind="ExternalInput")
    d_bkh = nc.dram_tensor("sa_bk_h", [256], F32, kind="ExternalInput")
    cnames = ["ca_wq", "ca_wk", "ca_wv", "ca_wo"]
    d_w = {n: nc.dram_tensor(n, [D, D], BF16, kind="ExternalInput")
           for n in cnames}
    d_w1 = nc.dram_tensor("f_w1", [D, F], BF16, kind="ExternalInput")
    d_w2 = nc.dram_tensor("f_w2", [F, D], BF16, kind="ExternalInput")
    d_bq_ca = nc.dram_tensor("ca_bq", [D], F32, kind="ExternalInput")
    d_bk_ca = nc.dram_tensor("ca_bk", [D], F32, kind="ExternalInput")
    d_b1 = nc.dram_tensor("f_b1", [F], F32, kind="ExternalInput")
    d_cvec = nc.dram_tensor("cvec", [D], BF16, kind="ExternalInput")
    d_b2v = nc.dram_tensor("b2v", [D], BF16, kind="ExternalInput")
    d_gbt = {n: nc.dram_tensor(n, [D], BF16, kind="ExternalInput")
             for n in ["sa_g", "sa_bt", "ca_g", "ca_bt", "f_g", "f_bt"]}
    d_out = nc.dram_tensor("out", [CH, D], F32, kind="ExternalOutput")

    cc = {
        "kt_in": nc.dram_tensor("cc_ca_kt_in", [D, CH], F8E3, kind="Internal"),
        "kt_out": nc.dram_tensor("cc_ca_kt_out", [4 * D, CH], F8E3,
                                 kind="Internal"),
        "v_in": nc.dram_tensor("cc_ca_v_in", [CH, D], F8E3, kind="Internal"),
        "v_out": nc.dram_tensor("cc_ca_v_out", [S, D], F8E3, kind="Internal"),
        "rs_in": nc.dram_tensor("cc_rs_in", [S, D], F8E3, kind="Internal"),
        "rs_out": nc.dram_tensor("cc_rs_out", [CH, D], F8E3, kind="Internal"),
    }
    GROUPS = [[0, 1, 2, 3], [4, 5, 6, 7]]

    with tile.TileContext(nc) as tc, ExitStack() as ctx:
        const = ctx.enter_context(tc.tile_pool(name="const", bufs=1))
        wpool = ctx.enter_context(tc.tile_pool(name="wpool", bufs=8))
        qpool = ctx.enter_context(tc.tile_pool(name="qpool", bufs=16))
        resp = ctx.enter_context(tc.tile_pool(name="resp", bufs=8))
        scrp = ctx.enter_context(tc.tile_pool(name="scrp", bufs=2))
        ps_s = ctx.enter_context(tc.tile_pool(name="ps_s", bufs=2, space="PSUM"))
        ps_av = ctx.enter_context(tc.tile_pool(name="ps_av", bufs=2, space="PSUM"))
        ps_m = ctx.enter_context(tc.tile_pool(name="ps_m", bufs=2, space="PSUM"))

        ident = const.tile([128, 128], F32, tag="ident")
        make_identity(nc, ident)
        identb = const.tile([128, 128], BF16, tag="identb")
        nc.vector.tensor_copy(identb, ident)
        onescol = const.tile([128, 1], BF16, tag="onescol")
        nc.vector.memset(onescol, 1.0)
        onesrow = const.tile([1, 64], F32, tag="onesrow")
        nc.vector.memset(onesrow, 1.0)
        epst = const.tile([128, 1], F32, tag="epst")
        nc.vector.memset(epst, EPS)
        zerot = const.tile([128, 1], F32, tag="zerot")
        nc.vector.memset(zerot, 0.0)

        def bias_cols(dram, ntiles, name):
            t = const.tile([128, ntiles], F32, tag=name, name=name)
            src = bass.AP(tensor=dram.ap().tensor, offset=0,
                          ap=[[1, 128], [128, ntiles]])
            nc.sync.dma_start(out=t, in_=src)
            return t

        def bcast_row(dram, tag, name):
            t = const.tile([128, D], BF16, tag=tag, bufs=2, name=name)
            src = bass.AP(tensor=dram.ap().tensor, offset=0, ap=[[0, 128], [1, D]])
            nc.sync.dma_start(out=t, in_=src)
            return t

        bq_h = bias_cols(d_bqh, 2, "bqh")
        bk_h = bias_cols(d_bkh, 2, "bkh")
        bq_ca = bias_cols(d_bq_ca, DT, "bqca")
        bk_ca = bias_cols(d_bk_ca, DT, "bkca")
        b1c = bias_cols(d_b1, NFT, "b1c")

        def layer_norm(src, g_t, bt_t, out):
            """[128, D] f32 LN along free dim; out may alias src."""
            stats = scrp.tile([128, 2, 6], F32, tag="lnstat", name="lnstat")
            for s in range(2):
                nc.vector.bn_stats(out=stats[:, s, :],
                                   in_=src[:, s * 512:(s + 1) * 512])
            mv = scrp.tile([128, 2], F32, tag="lnmv", name="lnmv")
            nc.vector.bn_aggr(out=mv, in_=stats)
            rstd = scrp.tile([128, 1], F32, tag="lnrstd", name="lnrstd")
            nc.scalar.activation(out=rstd, in_=mv[:, 1:2], func=AF.Sqrt,
                                 bias=epst, scale=1.0)
            nc.vector.reciprocal(out=rstd, in_=rstd)
            cent = scrp.tile([128, D], F32, tag="scr", name="cent")
            nc.vector.scalar_tensor_tensor(out=cent, in0=src, scalar=mv[:, 0:1],
                                           in1=g_t, op0=OP.subtract, op1=OP.mult)
            nc.vector.scalar_tensor_tensor(out=out, in0=cent, scalar=rstd,
                                           in1=bt_t, op0=OP.mult, op1=OP.add)

        def load_w8(wd, ncols=D):
            ws = []
            for k in range(DT):
                t = wpool.tile([128, ncols], BF16, tag="w", name=f"w_{k}")
                nc.sync.dma_start(out=t, in_=wd.ap()[k * 128:(k + 1) * 128, :])
                ws.append(t)
            return ws

        # =================== CA K/V local + AllGathers (first) ============
        with nc.named_scope("ca_kvlocal"):
            eq = []
            for k in range(DT):
                t = qpool.tile([128, CH], BF16, tag="qt", name=f"eq{k}")
                nc.sync.dma_start(out=t, in_=d_eTq.ap()[k * 128:(k + 1) * 128, :])
                eq.append(t)
            wk = load_w8(d_w["ca_wk"])
            for m in range(DT):
                ps = ps_m.tile([128, CH], F32, tag="ps_m", name="lkps")
                for k in range(DT):
                    nc.tensor.matmul(ps, wk[k][:, m * 128:(m + 1) * 128],
                                     eq[k], start=(k == 0), stop=(k == DT - 1))
                st = scrp.tile([128, CH], F8E3, tag="stage", bufs=4, name="ktst")
                nc.scalar.activation(out=st, in_=ps, func=AF.Identity,
                                     bias=bk_ca[:, m:m + 1], scale=1.0)
                nc.sync.dma_start(
                    out=cc["kt_in"].ap()[m * 128:(m + 1) * 128, :], in_=st)
            nc.gpsimd.collective_compute(
                "AllGather", mybir.AluOpType.bypass,
                ins=[cc["kt_in"].ap()], outs=[cc["kt_out"].ap()],
                replica_groups=GROUPS)
            wv = load_w8(d_w["ca_wv"])
            for tt in range(NMT):
                for n in range(2):
                    ps = ps_m.tile([128, CH], F32, tag="ps_m", name="lvps")
                    for k in range(DT):
                        nc.tensor.matmul(
                            ps, eq[k][:, tt * 128:(tt + 1) * 128],
                            wv[k][:, n * 512:(n + 1) * 512],
                            start=(k == 0), stop=(k == DT - 1))
                    st = scrp.tile([128, CH], F8E3, tag="stage", bufs=4,
                                   name="vst")
                    nc.scalar.activation(out=st, in_=ps, func=AF.Copy)
                    nc.sync.dma_start(
                        out=cc["v_in"].ap()[tt * 128:(tt + 1) * 128,
                                            n * 512:(n + 1) * 512],
                        in_=st)
            nc.gpsimd.collective_compute(
                "AllGather", mybir.AluOpType.bypass,
                ins=[cc["v_in"].ap()], outs=[cc["v_out"].ap()],
                replica_groups=GROUPS)

        # =================== SA local projections =========================
        with ExitStack() as sa_ctx:
            big = sa_ctx.enter_context(tc.tile_pool(name="big", bufs=2))
            maskp = sa_ctx.enter_context(tc.tile_pool(name="maskp", bufs=1))
            ppool = sa_ctx.enter_context(tc.tile_pool(name="ppool", bufs=3))
            zpool = sa_ctx.enter_context(tc.tile_pool(name="zpool", bufs=2))
            zsm = sa_ctx.enter_context(tc.tile_pool(name="zsm", bufs=2))

            # K^T and Q^T: [128 (2 heads x 64 dh), 2048 tokens] per pair
            KT = [big.tile([128, S], BF16, tag="KT", bufs=2, name=f"KT{g}")
                  for g in range(2)]
            QT = [big.tile([128, S], BF16, tag="QT", bufs=2, name=f"QT{g}")
                  for g in range(2)]
            # V: [128 tokens per tile, 16 tiles, 256 dh]  (bias folded)
            vt = big.tile([128, NST, 256], F8E3, tag="vt", bufs=1, name="vt")

            with ExitStack() as proj_ctx, nc.named_scope("sa_proj"):
                projp = proj_ctx.enter_context(
                    tc.tile_pool(name="projp", bufs=8))
                xt = []
                for k in range(DT):
                    t = projp.tile([128, S], BF16, tag="xt", bufs=8,
                                   name=f"xt{k}")
                    nc.sync.dma_start(out=t,
                                      in_=d_xT.ap()[k * 128:(k + 1) * 128, :])
                    xt.append(t)

                def load_wh(wd, tag):
                    ws = []
                    for k in range(DT):
                        t = projp.tile([128, 256], BF16, tag=tag, bufs=8,
                                       name=f"{tag}{k}")
                        nc.sync.dma_start(
                            out=t, in_=wd.ap()[k * 128:(k + 1) * 128, :])
                        ws.append(t)
                    return ws

                wqh = load_wh(d_wqh, "wqh")
                wkh = load_wh(d_wkh, "wkh")
                wvh = load_wh(d_wvh, "wvh")

                for g in range(2):
                    for tc4 in range(4):
                        ps = ps_m.tile([128, CH], F32, tag="ps_m", name="kps")
                        for k in range(DT):
                            nc.tensor.matmul(
                                ps, wkh[k][:, g * 128:(g + 1) * 128],
                                xt[k][:, tc4 * 512:(tc4 + 1) * 512],
                                start=(k == 0), stop=(k == DT - 1))
                        nc.scalar.activation(
                            out=KT[g][:, tc4 * 512:(tc4 + 1) * 512], in_=ps,
                            func=AF.Identity, bias=bk_h[:, g:g + 1], scale=1.0)
                for g in range(2):
                    for tc4 in range(4):
                        ps = ps_m.tile([128, CH], F32, tag="ps_m", name="qps")
                        for k in range(DT):
                            nc.tensor.matmul(
                                ps, wqh[k][:, g * 128:(g + 1) * 128],
                                xt[k][:, tc4 * 512:(tc4 + 1) * 512],
                                start=(k == 0), stop=(k == DT - 1))
                        nc.scalar.activation(
                            out=QT[g][:, tc4 * 512:(tc4 + 1) * 512], in_=ps,
                            func=AF.Identity, bias=bq_h[:, g:g + 1], scale=1.0)
                for tt in range(NST):
                    ps = ps_m.tile([128, 256], F32, tag="ps_m", name="vps")
                    for k in range(DT):
                        nc.tensor.matmul(
                            ps, xt[k][:, tt * 128:(tt + 1) * 128], wvh[k],
                            start=(k == 0), stop=(k == DT - 1))
                    nc.scalar.activation(out=vt[:, tt, :], in_=ps,
                                         func=AF.Copy)

            # ---- causal mask for the 4 diagonal tiles (same on all cores):
            # amask[k, dkt, q] = -240 if (128*dkt + k > q) else 0
            qmk = scrp.tile([128, CH], F32, tag="qmk", bufs=1, name="qmk")
            nc.gpsimd.iota(qmk, pattern=[[1, CH]], base=0,
                           channel_multiplier=-1,
                           allow_small_or_imprecise_dtypes=True)
            amask = maskp.tile([128, 4, CH], BF16, tag="mask", name="amask")
            for dkt in range(4):
                nc.vector.tensor_scalar(out=amask[:, dkt, :], in0=qmk,
                                        scalar1=float(128 * dkt),
                                        scalar2=-240.0, op0=OP.is_lt,
                                        op1=OP.mult)

            # =================== SA attention pairs =======================
            # qc-outer: after both pair-groups of a 512-token chunk finish,
            # its WO partial is computed and chunk-ReduceScatter qc fires --
            # RS 0..2 fly under the remaining pairs, only RS 3 is exposed.
            attnT = [big.tile([128, S], BF16, tag="atn", bufs=2,
                              name=f"atn{g}") for g in range(2)]
            woh = [big.tile([128, D], BF16, tag="woh", bufs=2,
                            name=f"woh{g}") for g in range(2)]
            for g in range(2):
                nc.sync.dma_start(
                    out=woh[g], in_=d_woh.ap()[g * 128:(g + 1) * 128, :])
            GPN = {0: 0, 1: 1, 2: 1, 3: 1}   # trailing kt2 iters on GpSimd
            for qc in (3, 2, 1, 0):
                for g in range(2):
                    with nc.named_scope(f"sa_g{g}q{qc}"):
                        nkt = 4 * qc + 4
                        nkt2 = nkt // 2
                        ngp = GPN[qc]
                        ndv = nkt2 - ngp
                        qa = QT[g][0:64, qc * 512:(qc + 1) * 512]
                        qb = QT[g][64:128, qc * 512:(qc + 1) * 512]
                        pav = ps_av.tile([128, CH], F32, tag="ps_av", bufs=2,
                                         name="pav")
                        zacc_d = zpool.tile([128, 2 * CH], BF16, tag="zd",
                                            bufs=2, name="zacc_d")
                        zacc_g = zpool.tile([128, 2 * CH], BF16, tag="zg",
                                            bufs=2, name="zacc_g")
                        for kt2 in range(nkt2):
                            pt2 = ppool.tile([128, 2, 2 * CH], BF16, tag="pt",
                                             name="pt")
                            for sub in range(2):
                                kt = 2 * kt2 + sub
                                diag = kt >= 4 * qc
                                pss = ps_s.tile([128, 2 * CH], F32, tag="ps_s",
                                                name="pss")
                                ksl = KT[g][:, kt * 128:(kt + 1) * 128]
                                nc.tensor.matmul(pss[:, 0:CH], ksl[0:64, :],
                                                 qa, start=True,
                                                 stop=not diag)
                                nc.tensor.matmul(pss[:, CH:2 * CH],
                                                 ksl[64:128, :], qb,
                                                 start=True, stop=True)
                                if diag:
                                    dkt = kt - 4 * qc
                                    nc.tensor.matmul(pss[:, 0:CH], identb,
                                                     amask[:, dkt, :],
                                                     start=False, stop=True,
                                                     skip_group_check=True)
                                nc.scalar.activation(
                                    out=pt2[:, sub, :], in_=pss, func=AF.Exp,
                                    bias=zerot, scale=1.0 / np.sqrt(DH))
                                if diag:
                                    dkt = kt - 4 * qc
                                    nc.vector.scalar_tensor_tensor(
                                        out=pt2[:, sub, CH:2 * CH],
                                        in0=amask[:, dkt, :],
                                        scalar=-1.0, op0=OP.is_ge,
                                        in1=pt2[:, sub, CH:2 * CH],
                                        op1=OP.mult)
                            if kt2 < ndv:
                                eng, zt, first = nc.vector, zacc_d, kt2 == 0
                            else:
                                eng, zt, first = nc.gpsimd, zacc_g, kt2 == ndv
                            if first:
                                eng.tensor_copy(zt, pt2[:, 0, :])
                            else:
                                eng.tensor_add(zt, zt, pt2[:, 0, :])
                            eng.tensor_add(zt, zt, pt2[:, 1, :])
                            for sub in range(2):
                                kt = 2 * kt2 + sub
                                vsl = vt[:, kt, g * 128:(g + 1) * 128]
                                nc.tensor.matmul(pav[0:64, :], vsl[:, 0:64],
                                                 pt2[:, sub, 0:CH],
                                                 start=(kt == 0),
                                                 stop=(kt == nkt - 1))
                                nc.tensor.matmul(pav[64:128, :],
                                                 vsl[:, 64:128],
                                                 pt2[:, sub, CH:2 * CH],
                                                 start=(kt == 0),
                                                 stop=(kt == nkt - 1))
                        at = attnT[g][:, qc * 512:(qc + 1) * 512]
                        nc.vector.tensor_copy(at, pav)
                        zs = zsm.tile([1, 2 * CH], F32, tag="zs", bufs=2,
                                      name="zs")
                        for h2 in range(2):
                            zf = ps_m.tile([1, CH], F32, tag="ps_m", name="zf")
                            nc.tensor.matmul(zf, onescol,
                                             zacc_d[:, h2 * CH:(h2 + 1) * CH],
                                             start=True, stop=(ngp == 0))
                            if ngp:
                                nc.tensor.matmul(
                                    zf, onescol,
                                    zacc_g[:, h2 * CH:(h2 + 1) * CH],
                                    start=False, stop=True)
                            nc.vector.tensor_copy(zs[:, h2 * CH:(h2 + 1) * CH],
                                                  zf)
                        przU = ps_m.tile([128, CH], F32, tag="ps_m",
                                         name="przU")
                        nc.tensor.matmul(przU[0:64, :], onesrow[0:1, :],
                                         zs[:, 0:CH], start=True, stop=True,
                                         tile_position=(0, 0))
                        nc.tensor.matmul(przU[64:128, :], onesrow[0:1, :],
                                         zs[:, CH:2 * CH], start=True,
                                         stop=True, tile_position=(0, 64))
                        rec = zsm.tile([128, CH], F32, tag="rec", bufs=2,
                                       name="rec")
                        nc.vector.reciprocal_approx_fast(out=rec, in_=przU)
                        nc.vector.tensor_mul(at, at, rec)
                # ---- WO partial for this 512-token chunk + chunk RS ----
                with nc.named_scope(f"sa_wo{qc}"):
                    for mt in range(4 * qc, 4 * qc + 4):
                        st = scrp.tile([128, D], F8E3, tag="stage2", bufs=4,
                                       name="rsst")
                        for n in range(2):
                            ps = ps_m.tile([128, CH], F32, tag="ps_m",
                                           name="wops")
                            for g in range(2):
                                nc.tensor.matmul(
                                    ps, attnT[g][:, mt * 128:(mt + 1) * 128],
                                    woh[g][:, n * 512:(n + 1) * 512],
                                    start=(g == 0), stop=(g == 1))
                            if (mt + n) % 2 == 0:
                                nc.scalar.activation(
                                    out=st[:, n * 512:(n + 1) * 512], in_=ps,
                                    func=AF.Copy)
                            else:
                                nc.vector.tensor_copy(
                                    st[:, n * 512:(n + 1) * 512], ps)
                        nc.sync.dma_start(
                            out=cc["rs_in"].ap()[mt * 128:(mt + 1) * 128, :],
                            in_=st)
                    nc.gpsimd.collective_compute(
                        "ReduceScatter", mybir.AluOpType.add,
                        ins=[cc["rs_in"].ap()[qc * 512:(qc + 1) * 512, :]],
                        outs=[cc["rs_out"].ap()[qc * 128:(qc + 1) * 128, :]],
                        replica_groups=GROUPS)

        # =================== post-RS: x1 = LN(RS + res1b) =================
        def transposeT(x_tiles, out_tag):
            """4 [128, D] f32 token-major -> 8 [128, CH] bf16 feature-major."""
            outs = [qpool.tile([128, CH], BF16, tag="qt",
                               name=f"{out_tag}{i}") for i in range(DT)]
            for mt in range(NMT):
                for ft in range(DT):
                    pst = ps_m.tile([128, 128], F32, tag="ps_m", name="tps")
                    nc.tensor.transpose(
                        pst, x_tiles[mt][:, ft * 128:(ft + 1) * 128], ident)
                    nc.vector.tensor_copy(
                        outs[ft][:, mt * 128:(mt + 1) * 128], pst)
            return outs

        with nc.named_scope("sa_ln"):
            g1 = bcast_row(d_gbt["sa_g"], "gt", "g1")
            bt1 = bcast_row(d_gbt["sa_bt"], "gt", "bt1")
            x1 = [None] * NMT
            x1T = [qpool.tile([128, CH], BF16, tag="qt", name=f"x1T{i}")
                   for i in range(DT)]
            for mt in (3, 2, 1, 0):   # RS completion order
                rst = scrp.tile([128, D], F8E3, tag="rst", bufs=2, name="rst")
                nc.sync.dma_start(
                    out=rst, in_=cc["rs_out"].ap()[mt * 128:(mt + 1) * 128, :])
                r1t = scrp.tile([128, D], BF16, tag="r1t", bufs=2, name="r1t")
                nc.sync.dma_start(
                    out=r1t, in_=d_res1b.ap()[mt * 128:(mt + 1) * 128, :])
                pre = resp.tile([128, D], F32, tag="persist", name=f"x1_{mt}")
                nc.vector.tensor_add(pre, rst, r1t)
                layer_norm(pre, g1, bt1, pre)
                x1[mt] = pre
                for ft in range(DT):
                    pst = ps_m.tile([128, 128], F32, tag="ps_m", name="tps")
                    nc.tensor.transpose(
                        pst, pre[:, ft * 128:(ft + 1) * 128], ident)
                    nc.vector.tensor_copy(
                        x1T[ft][:, mt * 128:(mt + 1) * 128], pst)

        # =================== CA attention =================================
        with ExitStack() as ca_ctx:
            kvp = ca_ctx.enter_context(tc.tile_pool(name="kvp", bufs=3))
            vpp = ca_ctx.enter_context(tc.tile_pool(name="vpp", bufs=1))
            ppool2 = ca_ctx.enter_context(tc.tile_pool(name="ppool2", bufs=4))
            zpool2 = ca_ctx.enter_context(tc.tile_pool(name="zpool2", bufs=2))
            attp = ca_ctx.enter_context(tc.tile_pool(name="attp", bufs=8))
            zsm2 = ca_ctx.enter_context(tc.tile_pool(name="zsm2", bufs=2))

            with nc.named_scope("ca_q"):
                wqc = load_w8(d_w["ca_wq"])
                QTca = []
                for m in range(DT):
                    ps = ps_m.tile([128, CH], F32, tag="ps_m", name="cqps")
                    for k in range(DT):
                        nc.tensor.matmul(ps, wqc[k][:, m * 128:(m + 1) * 128],
                                         x1T[k], start=(k == 0),
                                         stop=(k == DT - 1))
                    o = qpool.tile([128, CH], BF16, tag="qt", name=f"qca{m}")
                    nc.scalar.activation(out=o, in_=ps, func=AF.Identity,
                                         bias=bq_ca[:, m:m + 1], scale=1.0)
                    QTca.append(o)

            aun = []
            vcur = None
            for hp in range(H // 2):
                with nc.named_scope(f"ca_pair{hp}"):
                    # K^T for this pair straight from the fp8e3 AG buffer
                    ktp = kvp.tile([128, 4, CH], F8E3, tag="ktp", name="ktp")
                    nc.sync.dma_start(
                        out=ktp,
                        in_=bass.AP(tensor=cc["kt_out"].ap().tensor,
                                    offset=128 * hp * CH,
                                    ap=[[CH, 128], [D * CH, 4], [1, CH]]))
                    ktb = ktp.rearrange("p a q -> p (a q)")
                    if hp % 2 == 0:
                        vte = vpp.tile([128, NKT, 256], F8E3, tag="vpp",
                                       bufs=1, name="vpp")
                        nc.sync.dma_start(
                            out=vte,
                            in_=bass.AP(tensor=cc["v_out"].ap().tensor,
                                        offset=(hp // 2) * 256,
                                        ap=[[D, 128], [128 * D, NKT],
                                            [1, 256]]))
                        vcur = vte
                    voff = (hp % 2) * 128

                    qa = QTca[hp][0:64, :]
                    qb = QTca[hp][64:128, :]
                    pav = ps_av.tile([128, CH], F32, tag="ps_av", bufs=2,
                                     name="cpav")
                    zacc_d = zpool2.tile([128, 2 * CH], BF16, tag="zd",
                                         bufs=2, name="czd")
                    zacc_g = zpool2.tile([128, 2 * CH], BF16, tag="zg",
                                         bufs=2, name="czg")
                    NDV = 6   # kt2 0..5 on DVE, 6..7 on GpSimd
                    for kt2 in range(NKT // 2):
                        pt2 = ppool2.tile([128, 2, 2 * CH], BF16, tag="pt",
                                          name="cpt")
                        for sub in range(2):
                            kt = 2 * kt2 + sub
                            pss = ps_s.tile([128, 2 * CH], F32, tag="ps_s",
                                            name="cpss")
                            ksl = ktb[:, kt * 128:(kt + 1) * 128]
                            nc.tensor.matmul(pss[:, 0:CH], ksl[0:64, :], qa,
                                             start=True, stop=True)
                            nc.tensor.matmul(pss[:, CH:2 * CH],
                                             ksl[64:128, :], qb,
                                             start=True, stop=True)
                            nc.scalar.activation(
                                out=pt2[:, sub, :], in_=pss, func=AF.Exp,
                                bias=zerot, scale=1.0 / np.sqrt(DH))
                        if kt2 < NDV:
                            eng, zt, first = nc.vector, zacc_d, kt2 == 0
                        else:
                            eng, zt, first = nc.gpsimd, zacc_g, kt2 == NDV
                        if first:
                            eng.tensor_copy(zt, pt2[:, 0, :])
                        else:
                            eng.tensor_add(zt, zt, pt2[:, 0, :])
                        eng.tensor_add(zt, zt, pt2[:, 1, :])
                        for sub in range(2):
                            kt = 2 * kt2 + sub
                            vsl = vcur[:, kt, voff:voff + 128]
                            nc.tensor.matmul(pav[0:64, :], vsl[:, 0:64],
                                             pt2[:, sub, 0:CH],
                                             start=(kt == 0),
                                             stop=(kt == NKT - 1))
                            nc.tensor.matmul(pav[64:128, :], vsl[:, 64:128],
                                             pt2[:, sub, CH:2 * CH],
                                             start=(kt == 0),
                                             stop=(kt == NKT - 1))
                    at = attp.tile([128, CH], BF16, tag="aun", name=f"aun{hp}")
                    nc.vector.tensor_copy(at, pav)
                    aun.append(at)
                    zs = zsm2.tile([1, 2 * CH], F32, tag="zs", bufs=2,
                                   name="czs")
                    for h2 in range(2):
                        zf = ps_m.tile([1, CH], F32, tag="ps_m", name="czf")
                        nc.tensor.matmul(zf, onescol,
                                         zacc_d[:, h2 * CH:(h2 + 1) * CH],
                                         start=True, stop=False)
                        nc.tensor.matmul(zf, onescol,
                                         zacc_g[:, h2 * CH:(h2 + 1) * CH],
                                         start=False, stop=True)
                        nc.vector.tensor_copy(zs[:, h2 * CH:(h2 + 1) * CH], zf)
                    przU = ps_m.tile([128, CH], F32, tag="ps_m", name="cprz")
                    nc.tensor.matmul(przU[0:64, :], onesrow[0:1, :],
                                     zs[:, 0:CH], start=True, stop=True,
                                     tile_position=(0, 0))
                    nc.tensor.matmul(przU[64:128, :], onesrow[0:1, :],
                                     zs[:, CH:2 * CH], start=True, stop=True,
                                     tile_position=(0, 64))
                    rec = zsm2.tile([128, CH], F32, tag="rec", bufs=2,
                                    name="crec")
                    nc.vector.reciprocal_approx_fast(out=rec, in_=przU)
                    nc.vector.tensor_mul(at, at, rec)

            # ---- CA WO + residual(x1) + cvec + LN -> y1, y1T ----
            with nc.named_scope("ca_wo_ln"):
                cvec_t = bcast_row(d_cvec, "vec", "cvec")
                g2 = bcast_row(d_gbt["ca_g"], "gt", "g2")
                bt2 = bcast_row(d_gbt["ca_bt"], "gt", "bt2")
                wo = load_w8(d_w["ca_wo"])
                y1 = []
                for mt in range(NMT):
                    pre = resp.tile([128, D], F32, tag="persist",
                                    name=f"y1_{mt}")
                    for n in range(2):
                        ps = ps_m.tile([128, 512], F32, tag="ps_m",
                                       name="cwops")
                        for k in range(DT):
                            nc.tensor.matmul(
                                ps, aun[k][:, mt * 128:(mt + 1) * 128],
                                wo[k][:, n * 512:(n + 1) * 512],
                                start=(k == 0), stop=(k == DT - 1))
                        nc.vector.tensor_add(pre[:, n * 512:(n + 1) * 512],
                                             ps,
                                             x1[mt][:, n * 512:(n + 1) * 512])
                    nc.vector.tensor_add(pre, pre, cvec_t)
                    layer_norm(pre, g2, bt2, pre)
                    y1.append(pre)
                y1T = transposeT(y1, "y1T")

        # =================== FFN ==========================================
        with ExitStack() as ffn_ctx:
            hpool = ffn_ctx.enter_context(tc.tile_pool(name="hpool", bufs=32))
            w1pool = ffn_ctx.enter_context(tc.tile_pool(name="w1pool", bufs=8))
            with nc.named_scope("ffn1"):
                w1 = []
                for k in range(DT):
                    t = w1pool.tile([128, F], BF16, tag="w1", name=f"w1_{k}")
                    nc.sync.dma_start(out=t,
                                      in_=d_w1.ap()[k * 128:(k + 1) * 128, :])
                    w1.append(t)
                hT = []
                for m in range(NFT):
                    ps = ps_m.tile([128, CH], F32, tag="ps_m", name="f1ps")
                    for k in range(DT):
                        nc.tensor.matmul(ps, w1[k][:, m * 128:(m + 1) * 128],
                                         y1T[k], start=(k == 0),
                                         stop=(k == DT - 1))
                    h = hpool.tile([128, CH], BF16, tag="h", name=f"h{m}")
                    nc.scalar.activation(out=h, in_=ps, func=AF.Relu,
                                         bias=b1c[:, m:m + 1], scale=1.0)
                    hT.append(h)
            with nc.named_scope("ffn2"):
                b2v_t = bcast_row(d_b2v, "vec", "b2v")
                h2 = [resp.tile([128, D], F32, tag="persist", name=f"h2_{i}")
                      for i in range(NMT)]
                for n in range(2):
                    pss = [ps_s.tile([128, 2 * CH], F32, tag="ps_s",
                                     name=f"f2ps{n}_{i}") for i in range(2)]
                    for kb in range(4):
                        w2b = w1pool.tile([128, 8, 512], BF16, tag="w1",
                                          name=f"w2b{kb}")
                        nc.sync.dma_start(
                            out=w2b,
                            in_=bass.AP(tensor=d_w2.ap().tensor,
                                        offset=kb * 8 * 128 * D + n * 512,
                                        ap=[[D, 128], [128 * D, 8], [1, 512]]))
                        for ks in range(8):
                            k = kb * 8 + ks
                            for mt in range(NMT):
                                nc.tensor.matmul(
                                    pss[mt // 2][:, (mt % 2) * CH:
                                                 (mt % 2 + 1) * CH],
                                    hT[k][:, mt * 128:(mt + 1) * 128],
                                    w2b[:, ks, :],
                                    start=(k == 0), stop=(k == NFT - 1))
                    for mt in range(NMT):
                        nc.vector.tensor_add(
                            h2[mt][:, n * 512:(n + 1) * 512],
                            pss[mt // 2][:, (mt % 2) * CH:(mt % 2 + 1) * CH],
                            y1[mt][:, n * 512:(n + 1) * 512])
            with nc.named_scope("ln3_out"):
                g3 = bcast_row(d_gbt["f_g"], "gt", "g3")
                bt3 = bcast_row(d_gbt["f_bt"], "gt", "bt3")
                for mt in range(NMT):
                    nc.vector.tensor_add(h2[mt], h2[mt], b2v_t)
                    layer_norm(h2[mt], g3, bt3, h2[mt])
                    nc.sync.dma_start(out=d_out.ap()[mt * 128:(mt + 1) * 128, :],
                                      in_=h2[mt])

    nc.compile()
    return nc


def _bf(a):
    return np.ascontiguousarray(a, dtype=np.float32).astype(ml_dtypes.bfloat16)


def kernel(**inputs):
    global _CACHED
    if _CACHED is None:
        _CACHED = build()
    nc = _CACHED

    f = {k: np.asarray(v, dtype=np.float32) for k, v in inputs.items()}
    dec, enc = f["decoder_input"], f["encoder_output"]
    cvec = (f["ca_bv"] @ f["ca_wo"] + f["ca_bo"]).astype(np.float32)
    r1vec = (f["sa_bv"] @ f["sa_wo"] + f["sa_bo"]).astype(np.float32)

    shared = {n: _bf(f[n]) for n in
              ["ca_wq", "ca_wk", "ca_wv", "ca_wo", "f_w1", "f_w2"]}
    shared.update({n: f[n] for n in ["ca_bq", "ca_bk", "f_b1"]})
    shared["cvec"] = _bf(cvec)
    shared["b2v"] = _bf(f["f_b2"])
    for n in ["sa_g", "sa_bt", "ca_g", "ca_bt", "f_g", "f_bt"]:
        shared[n] = _bf(f[n])

    in_maps = []
    rows_of = {}
    for c in range(8):
        b, j = c // 4, c % 4
        rows = np.concatenate([np.arange(512 * qc + 128 * j,
                                         512 * qc + 128 * j + 128)
                               for qc in range(4)])
        rows_of[c] = rows
        hs = slice(j * 256, (j + 1) * 256)
        m = {
            "xT": _bf(dec[b].T),
            "eTq": _bf(enc[b, rows, :].T),
            "res1b": _bf(dec[b, rows, :] + r1vec),
            "sa_wq_h": _bf(f["sa_wq"][:, hs]),
            "sa_wk_h": _bf(f["sa_wk"][:, hs]),
            "sa_wv_h": _bf(f["sa_wv"][:, hs]),
            "sa_wo_h": _bf(f["sa_wo"][hs, :]),
            "sa_bq_h": np.ascontiguousarray(f["sa_bq"][hs], dtype=np.float32),
            "sa_bk_h": np.ascontiguousarray(f["sa_bk"][hs], dtype=np.float32),
        }
        m.update(shared)
        in_maps.append(m)

    global LAST_RES
    res = bass_utils.run_bass_kernel_spmd(nc, in_maps, core_ids=list(range(8)))
    LAST_RES = res
    out = np.empty((B, S, D), dtype=np.float32)
    for c in range(8):
        b = c // 4
        out[b, rows_of[c], :] = res.results[c]["out"]
    return out


# revision 10
# speedup vs baseline: 1.1077x; 1.0046x over previous
"""Transformer decoder block (self-attn + cross-attn + FFN, post-LN) on 8
Trainium2 NeuronCores.

v3: head-sharded causal self-attention + token-sharded cross-attn/FFN.

8 cores = 2 batches x 4 ranks. Rank j of a batch group:
  - SA: computes heads [4j, 4j+4) for ALL 2048 tokens. K/V/Q projected
    locally from the full decoder input (no collective before attention).
    Causal structure is uniform across cores: per 512-query chunk qc only
    key tiles kt < 4(qc+1) are computed (62.5% of the full score work) and
    only the 4 diagonal tiles are masked. WO partials [2048, 1024] are
    staged in fp8e3 and summed across the 4 ranks with two column-split
    ReduceScatters; rank j receives its own 512-token slice.
  - CA: token-sharded as v2 — each rank projects K/V for its 512 encoder
    tokens, one fused AllGather per tensor (fp8e3), attention for its 512
    queries over all 2048 keys. The CA AllGathers are issued early and fly
    under the SA compute.
  - FFN + all residual/LN paths: token-sharded (512 tokens per rank).

v3 micro-optimizations vs v2:
  - CA score/AV matmuls read the fp8 AllGather buffers directly as the
    stationary operand (mixed fp8xbf16 matmul) — the fp8->bf16 DVE casts
    are gone.
  - K/V transport in fp8e3 (e3m4) instead of e4m3: halves the
    quantization error of the collective path.
  - softmax 1/Z via reciprocal_approx_fast (~5x faster than reciprocal).
  - Z-accumulation split between the DVE and the (otherwise idle) GpSimd
    engine: two partial accumulators, merged by the partition-sum matmul.
  - residual 1 (decoder input + bv@wo+bo) precomputed host-side and DMAd
    in [token, feature] layout directly (no PE transposes to rebuild it).
All matmuls bf16 (or fp8e3 stationary) with fp32 PSUM accumulation.
"""

from contextlib import ExitStack

import numpy as np
import ml_dtypes

import concourse.bass as bass
import concourse.bacc as bacc
import concourse.mybir as mybir
import concourse.tile as tile
from concourse import bass_utils
from concourse.masks import make_identity

BF16 = mybir.dt.bfloat16
F8E3 = mybir.dt.float8e3
F32 = mybir.dt.float32
AF = mybir.ActivationFunctionType
OP = mybir.AluOpType

B, S, D, H, F = 2, 2048, 1024, 16, 4096
DH = 64
EPS = 1e-5
CH = 512          # output tokens per core
DT = D // 128     # 8 feature tiles
NKT = S // 128    # 16 key tiles
NMT = CH // 128   # 4 token tiles per core (output)
NFT = F // 128    # 32 FFN hidden tiles
NST = S // 128    # 16 token tiles (full sequence)

_CACHED = None


def build():
    nc = bacc.Bacc("TRN2", target_bir_lowering=False, debug=False,
                   enable_asserts=False, num_devices=8)

    # ---- per-core DRAM I/O ----
    d_xT = nc.dram_tensor("xT", [D, S], BF16, kind="ExternalInput")
    d_eTq = nc.dram_tensor("eTq", [D, CH], BF16, kind="ExternalInput")
    d_res1b = nc.dram_tensor("res1b", [CH, D], BF16, kind="ExternalInput")
    d_wqh = nc.dram_tensor("sa_wq_h", [D, 256], BF16, kind="ExternalInput")
    d_wkh = nc.dram_tensor("sa_wk_h", [D, 256], BF16, kind="ExternalInput")
    d_wvh = nc.dram_tensor("sa_wv_h", [D, 256], BF16, kind="ExternalInput")
    d_woh = nc.dram_tensor("sa_wo_h", [256, D], BF16, kind="ExternalInput")
    d_bqh = nc.dram_tensor("sa_bq_h", [256], F32, kind="ExternalInput")
    d_bkh = nc.dram_tensor("sa_bk_h", [256], F32, kind="ExternalInput")
    cnames = ["ca_wq", "ca_wk", "ca_wv", "ca_wo"]
    d_w = {n: nc.dram_tensor(n, [D, D], BF16, kind="ExternalInput")
           for n in cnames}
    d_w1 = nc.dram_tensor("f_w1", [D, F], BF16, kind="ExternalInput")
    d_w2 = nc.dram_tensor("f_w2", [F, D], BF16, kind="ExternalInput")
    d_bq_ca = nc.dram_tensor("ca_bq", [D], F32, kind="ExternalInput")
    d_bk_ca = nc.dram_tensor("ca_bk", [D], F32, kind="ExternalInput")
    d_b1 = nc.dram_tensor("f_b1", [F], F32, kind="ExternalInput")
    d_cvec = nc.dram_tensor("cvec", [D], BF16, kind="ExternalInput")
    d_b2v = nc.dram_tensor("b2v", [D], BF16, kind="ExternalInput")
    d_gbt = {n: nc.dram_tensor(n, [D], BF16, kind="ExternalInput")
             for n in ["sa_g", "sa_bt", "ca_g", "ca_bt", "f_g", "f_bt"]}
    d_out = nc.dram_tensor("out", [CH, D], F32, kind="ExternalOutput")

    cc = {
        "kt_in": nc.dram_tensor("cc_ca_kt_in", [D, CH], F8E3, kind="Internal"),
        "kt_out": nc.dram_tensor("cc_ca_kt_out", [4 * D, CH], F8E3,
                                 kind="Internal"),
        "v_in": nc.dram_tensor("cc_ca_v_in", [CH, D], F8E3, kind="Internal"),
        "v_out": nc.dram_tensor("cc_ca_v_out", [S, D], F8E3, kind="Internal"),
        "rs_in": nc.dram_tensor("cc_rs_in", [S, D], F8E3, kind="Internal"),
        "rs_out": nc.dram_tensor("cc_rs_out", [CH, D], F8E3, kind="Internal"),
    }
    GROUPS = [[0, 1, 2, 3], [4, 5, 6, 7]]

    with tile.TileContext(nc) as tc, ExitStack() as ctx:
        const = ctx.enter_context(tc.tile_pool(name="const", bufs=1))
        wpool = ctx.enter_context(tc.tile_pool(name="wpool", bufs=8))
        qpool = ctx.enter_context(tc.tile_pool(name="qpool", bufs=16))
        resp = ctx.enter_context(tc.tile_pool(name="resp", bufs=8))
        scrp = ctx.enter_context(tc.tile_pool(name="scrp", bufs=2))
        ps_s = ctx.enter_context(tc.tile_pool(name="ps_s", bufs=2, space="PSUM"))
        ps_av = ctx.enter_context(tc.tile_pool(name="ps_av", bufs=2, space="PSUM"))
        ps_m = ctx.enter_context(tc.tile_pool(name="ps_m", bufs=2, space="PSUM"))

        ident = const.tile([128, 128], F32, tag="ident")
        make_identity(nc, ident)
        identb = const.tile([128, 128], BF16, tag="identb")
        nc.vector.tensor_copy(identb, ident)
        onescol = const.tile([128, 1], BF16, tag="onescol")
        nc.vector.memset(onescol, 1.0)
        onesrow = const.tile([1, 64], F32, tag="onesrow")
        nc.vector.memset(onesrow, 1.0)
        epst = const.tile([128, 1], F32, tag="epst")
        nc.vector.memset(epst, EPS)
        zerot = const.tile([128, 1], F32, tag="zerot")
        nc.vector.memset(zerot, 0.0)

        def bias_cols(dram, ntiles, name):
            t = const.tile([128, ntiles], F32, tag=name, name=name)
            src = bass.AP(tensor=dram.ap().tensor, offset=0,
                          ap=[[1, 128], [128, ntiles]])
            nc.sync.dma_start(out=t, in_=src)
            return t

        def bcast_row(dram, tag, name):
            t = const.tile([128, D], BF16, tag=tag, bufs=2, name=name)
            src = bass.AP(tensor=dram.ap().tensor, offset=0, ap=[[0, 128], [1, D]])
            nc.sync.dma_start(out=t, in_=src)
            return t

        bq_h = bias_cols(d_bqh, 2, "bqh")
        bk_h = bias_cols(d_bkh, 2, "bkh")
        bq_ca = bias_cols(d_bq_ca, DT, "bqca")
        bk_ca = bias_cols(d_bk_ca, DT, "bkca")
        b1c = bias_cols(d_b1, NFT, "b1c")

        def layer_norm(src, g_t, bt_t, out):
            """[128, D] f32 LN along free dim; out may alias src."""
            stats = scrp.tile([128, 2, 6], F32, tag="lnstat", name="lnstat")
            for s in range(2):
                nc.vector.bn_stats(out=stats[:, s, :],
                                   in_=src[:, s * 512:(s + 1) * 512])
            mv = scrp.tile([128, 2], F32, tag="lnmv", name="lnmv")
            nc.vector.bn_aggr(out=mv, in_=stats)
            rstd = scrp.tile([128, 1], F32, tag="lnrstd", name="lnrstd")
            nc.scalar.activation(out=rstd, in_=mv[:, 1:2], func=AF.Sqrt,
                                 bias=epst, scale=1.0)
            nc.vector.reciprocal(out=rstd, in_=rstd)
            cent = scrp.tile([128, D], F32, tag="scr", name="cent")
            nc.vector.scalar_tensor_tensor(out=cent, in0=src, scalar=mv[:, 0:1],
                                           in1=g_t, op0=OP.subtract, op1=OP.mult)
            nc.vector.scalar_tensor_tensor(out=out, in0=cent, scalar=rstd,
                                           in1=bt_t, op0=OP.mult, op1=OP.add)

        def load_w8(wd, ncols=D):
            ws = []
            for k in range(DT):
                t = wpool.tile([128, ncols], BF16, tag="w", name=f"w_{k}")
                nc.sync.dma_start(out=t, in_=wd.ap()[k * 128:(k + 1) * 128, :])
                ws.append(t)
            return ws

        # =================== CA K/V local + AllGathers (first) ============
        with nc.named_scope("ca_kvlocal"):
            eq = []
            for k in range(DT):
                t = qpool.tile([128, CH], BF16, tag="qt", name=f"eq{k}")
                nc.sync.dma_start(out=t, in_=d_eTq.ap()[k * 128:(k + 1) * 128, :])
                eq.append(t)
            wk = load_w8(d_w["ca_wk"])
            for m in range(DT):
                ps = ps_m.tile([128, CH], F32, tag="ps_m", name="lkps")
                for k in range(DT):
                    nc.tensor.matmul(ps, wk[k][:, m * 128:(m + 1) * 128],
                                     eq[k], start=(k == 0), stop=(k == DT - 1))
                st = scrp.tile([128, CH], F8E3, tag="stage", bufs=4, name="ktst")
                nc.scalar.activation(out=st, in_=ps, func=AF.Identity,
                                     bias=bk_ca[:, m:m + 1], scale=1.0)
                nc.sync.dma_start(
                    out=cc["kt_in"].ap()[m * 128:(m + 1) * 128, :], in_=st)
            nc.gpsimd.collective_compute(
                "AllGather", mybir.AluOpType.bypass,
                ins=[cc["kt_in"].ap()], outs=[cc["kt_out"].ap()],
                replica_groups=GROUPS)
            wv = load_w8(d_w["ca_wv"])
            for tt in range(NMT):
                for n in range(2):
                    ps = ps_m.tile([128, CH], F32, tag="ps_m", name="lvps")
                    for k in range(DT):
                        nc.tensor.matmul(
                            ps, eq[k][:, tt * 128:(tt + 1) * 128],
                            wv[k][:, n * 512:(n + 1) * 512],
                            start=(k == 0), stop=(k == DT - 1))
                    st = scrp.tile([128, CH], F8E3, tag="stage", bufs=4,
                                   name="vst")
                    nc.scalar.activation(out=st, in_=ps, func=AF.Copy)
                    nc.sync.dma_start(
                        out=cc["v_in"].ap()[tt * 128:(tt + 1) * 128,
                                            n * 512:(n + 1) * 512],
                        in_=st)
            nc.gpsimd.collective_compute(
                "AllGather", mybir.AluOpType.bypass,
                ins=[cc["v_in"].ap()], outs=[cc["v_out"].ap()],
                replica_groups=GROUPS)

        # =================== SA local projections =========================
        with ExitStack() as sa_ctx:
            big = sa_ctx.enter_context(tc.tile_pool(name="big", bufs=2))
            maskp = sa_ctx.enter_context(tc.tile_pool(name="maskp", bufs=1))
            ppool = sa_ctx.enter_context(tc.tile_pool(name="ppool", bufs=3))
            zpool = sa_ctx.enter_context(tc.tile_pool(name="zpool", bufs=2))
            zsm = sa_ctx.enter_context(tc.tile_pool(name="zsm", bufs=2))

            # K^T and Q^T: [128 (2 heads x 64 dh), 2048 tokens] per pair
            KT = [big.tile([128, S], BF16, tag="KT", bufs=2, name=f"KT{g}")
                  for g in range(2)]
            QT = [big.tile([128, S], BF16, tag="QT", bufs=2, name=f"QT{g}")
                  for g in range(2)]
            # V: [128 tokens per tile, 16 tiles, 256 dh]  (bias folded)
            vt = big.tile([128, NST, 256], F8E3, tag="vt", bufs=1, name="vt")

            with ExitStack() as proj_ctx, nc.named_scope("sa_proj"):
                projp = proj_ctx.enter_context(
                    tc.tile_pool(name="projp", bufs=8))
                xt = []
                for k in range(DT):
                    t = projp.tile([128, S], BF16, tag="xt", bufs=8,
                                   name=f"xt{k}")
                    nc.sync.dma_start(out=t,
                                      in_=d_xT.ap()[k * 128:(k + 1) * 128, :])
                    xt.append(t)

                def load_wh(wd, tag):
                    ws = []
                    for k in range(DT):
                        t = projp.tile([128, 256], BF16, tag=tag, bufs=8,
                                       name=f"{tag}{k}")
                        nc.sync.dma_start(
                            out=t, in_=wd.ap()[k * 128:(k + 1) * 128, :])
                        ws.append(t)
                    return ws

                wqh = load_wh(d_wqh, "wqh")
                wkh = load_wh(d_wkh, "wkh")
                wvh = load_wh(d_wvh, "wvh")

                for g in range(2):
                    for tc4 in range(4):
                        ps = ps_m.tile([128, CH], F32, tag="ps_m", name="kps")
                        for k in range(DT):
                            nc.tensor.matmul(
                                ps, wkh[k][:, g * 128:(g + 1) * 128],
                                xt[k][:, tc4 * 512:(tc4 + 1) * 512],
                                start=(k == 0), stop=(k == DT - 1))
                        nc.scalar.activation(
                            out=KT[g][:, tc4 * 512:(tc4 + 1) * 512], in_=ps,
                            func=AF.Identity, bias=bk_h[:, g:g + 1], scale=1.0)
                for g in range(2):
                    for tc4 in range(4):
                        ps = ps_m.tile([128, CH], F32, tag="ps_m", name="qps")
                        for k in range(DT):
                            nc.tensor.matmul(
                                ps, wqh[k][:, g * 128:(g + 1) * 128],
                                xt[k][:, tc4 * 512:(tc4 + 1) * 512],
                                start=(k == 0), stop=(k == DT - 1))
                        nc.scalar.activation(
                            out=QT[g][:, tc4 * 512:(tc4 + 1) * 512], in_=ps,
                            func=AF.Identity, bias=bq_h[:, g:g + 1], scale=1.0)
                for tt in range(NST):
                    ps = ps_m.tile([128, 256], F32, tag="ps_m", name="vps")
                    for k in range(DT):
                        nc.tensor.matmul(
                            ps, xt[k][:, tt * 128:(tt + 1) * 128], wvh[k],
                            start=(k == 0), stop=(k == DT - 1))
                    nc.scalar.activation(out=vt[:, tt, :], in_=ps,
                                         func=AF.Copy)

            # ---- causal mask for the 4 diagonal tiles (same on all cores):
            # amask[k, dkt, q] = -240 if (128*dkt + k > q) else 0
            qmk = scrp.tile([128, CH], F32, tag="qmk", bufs=1, name="qmk")
            nc.gpsimd.iota(qmk, pattern=[[1, CH]], base=0,
                           channel_multiplier=-1,
                           allow_small_or_imprecise_dtypes=True)
            amask = maskp.tile([128, 4, CH], BF16, tag="mask", name="amask")
            for dkt in range(4):
                nc.vector.tensor_scalar(out=amask[:, dkt, :], in0=qmk,
                                        scalar1=float(128 * dkt),
                                        scalar2=-240.0, op0=OP.is_lt,
                                        op1=OP.mult)

            # =================== SA attention pairs =======================
            # qc-outer: after both pair-groups of a 512-token chunk finish,
            # its WO partial is computed and chunk-ReduceScatter qc fires --
            # RS 0..2 fly under the remaining pairs, only RS 3 is exposed.
            attnT = [big.tile([128, S], BF16, tag="atn", bufs=2,
                              name=f"atn{g}") for g in range(2)]
            woh = [big.tile([128, D], BF16, tag="woh", bufs=2,
                            name=f"woh{g}") for g in range(2)]
            for g in range(2):
                nc.sync.dma_start(
                    out=woh[g], in_=d_woh.ap()[g * 128:(g + 1) * 128, :])
            GPN = {0: 0, 1: 0, 2: 1, 3: 1}   # trailing kt2 iters on GpSimd

            def z_finisher(zacc_d, zacc_g, ngp, at):
                def fin():
                    zs = zsm.tile([1, 2 * CH], F32, tag="zs", bufs=2,
                                  name="zs")
                    for h2 in range(2):
                        zf = ps_m.tile([1, CH], F32, tag="ps_m", name="zf")
                        nc.tensor.matmul(zf, onescol,
                                         zacc_d[:, h2 * CH:(h2 + 1) * CH],
                                         start=True, stop=(ngp == 0))
                        if ngp:
                            nc.tensor.matmul(
                                zf, onescol,
                                zacc_g[:, h2 * CH:(h2 + 1) * CH],
                                start=False, stop=True)
                        nc.vector.tensor_copy(zs[:, h2 * CH:(h2 + 1) * CH],
                                              zf)
                    przU = ps_m.tile([128, CH], F32, tag="ps_m", name="przU")
                    nc.tensor.matmul(przU[0:64, :], onesrow[0:1, :],
                                     zs[:, 0:CH], start=True, stop=True,
                                     tile_position=(0, 0))
                    nc.tensor.matmul(przU[64:128, :], onesrow[0:1, :],
                                     zs[:, CH:2 * CH], start=True,
                                     stop=True, tile_position=(0, 64))
                    rec = zsm.tile([128, CH], F32, tag="rec", bufs=2,
                                   name="rec")
                    nc.vector.reciprocal_approx_fast(out=rec, in_=przU)
                    nc.vector.tensor_mul(at, at, rec)
                return fin

            # finishers run lagged by one pair-unit so the PE queue never
            # waits on the previous pair's Z chain.
            pending = []   # (unit_idx, closure)

            def drain(upto):
                while pending and pending[0][0] <= upto:
                    pending.pop(0)[1]()

            units = [(qc, g) for qc in (3, 2, 1, 0) for g in (0, 1)]
            for idx, (qc, g) in enumerate(units):
                with nc.named_scope(f"sa_g{g}q{qc}"):
                    nkt = 4 * qc + 4
                    nkt2 = nkt // 2
                    ngp = GPN[qc]
                    ndv = nkt2 - ngp
                    qa = QT[g][0:64, qc * 512:(qc + 1) * 512]
                    qb = QT[g][64:128, qc * 512:(qc + 1) * 512]
                    pav = ps_av.tile([128, CH], F32, tag="ps_av", bufs=2,
                                     name="pav")
                    zacc_d = zpool.tile([128, 2 * CH], BF16, tag="zd",
                                        bufs=2, name="zacc_d")
                    zacc_g = zpool.tile([128, 2 * CH], BF16, tag="zg",
                                        bufs=2, name="zacc_g")
                    for kt2 in range(nkt2):
                        pt2 = ppool.tile([128, 2, 2 * CH], BF16, tag="pt",
                                         name="pt")
                        for sub in range(2):
                            kt = 2 * kt2 + sub
                            diag = kt >= 4 * qc
                            pss = ps_s.tile([128, 2 * CH], F32, tag="ps_s",
                                            name="pss")
                            ksl = KT[g][:, kt * 128:(kt + 1) * 128]
                            nc.tensor.matmul(pss[:, 0:CH], ksl[0:64, :],
                                             qa, start=True,
                                             stop=not diag)
                            nc.tensor.matmul(pss[:, CH:2 * CH],
                                             ksl[64:128, :], qb,
                                             start=True, stop=True)
                            if diag:
                                dkt = kt - 4 * qc
                                nc.tensor.matmul(pss[:, 0:CH], identb,
                                                 amask[:, dkt, :],
                                                 start=False, stop=True,
                                                 skip_group_check=True)
                            nc.scalar.activation(
                                out=pt2[:, sub, :], in_=pss, func=AF.Exp,
                                bias=zerot, scale=1.0 / np.sqrt(DH))
                            if diag:
                                dkt = kt - 4 * qc
                                nc.vector.scalar_tensor_tensor(
                                    out=pt2[:, sub, CH:2 * CH],
                                    in0=amask[:, dkt, :],
                                    scalar=-1.0, op0=OP.is_ge,
                                    in1=pt2[:, sub, CH:2 * CH],
                                    op1=OP.mult)
                        if kt2 < ndv:
                            eng, zt, first = nc.vector, zacc_d, kt2 == 0
                        else:
                            eng, zt, first = nc.gpsimd, zacc_g, kt2 == ndv
                        if first:
                            eng.tensor_copy(zt, pt2[:, 0, :])
                        else:
                            eng.tensor_add(zt, zt, pt2[:, 0, :])
                        eng.tensor_add(zt, zt, pt2[:, 1, :])
                        for sub in range(2):
                            kt = 2 * kt2 + sub
                            vsl = vt[:, kt, g * 128:(g + 1) * 128]
                            nc.tensor.matmul(pav[0:64, :], vsl[:, 0:64],
                                             pt2[:, sub, 0:CH],
                                             start=(kt == 0),
                                             stop=(kt == nkt - 1))
                            nc.tensor.matmul(pav[64:128, :],
                                             vsl[:, 64:128],
                                             pt2[:, sub, CH:2 * CH],
                                             start=(kt == 0),
                                             stop=(kt == nkt - 1))
                    at = attnT[g][:, qc * 512:(qc + 1) * 512]
                    nc.vector.tensor_copy(at, pav)
                    pending.append((idx, z_finisher(zacc_d, zacc_g, ngp, at)))
                if g == 1:
                    def wo_rs(qc=qc):
                        with nc.named_scope(f"sa_wo{qc}"):
                            for mt in range(4 * qc, 4 * qc + 4):
                                st = scrp.tile([128, D], F8E3, tag="stage2",
                                               bufs=4, name="rsst")
                                for n in range(2):
                                    ps = ps_m.tile([128, CH], F32, tag="ps_m",
                                                   name="wops")
                                    for gg in range(2):
                                        nc.tensor.matmul(
                                            ps,
                                            attnT[gg][:, mt * 128:
                                                      (mt + 1) * 128],
                                            woh[gg][:, n * 512:(n + 1) * 512],
                                            start=(gg == 0), stop=(gg == 1))
                                    if (mt + n) % 2 == 0:
                                        nc.scalar.activation(
                                            out=st[:, n * 512:(n + 1) * 512],
                                            in_=ps, func=AF.Copy)
                                    else:
                                        nc.vector.tensor_copy(
                                            st[:, n * 512:(n + 1) * 512], ps)
                                nc.sync.dma_start(
                                    out=cc["rs_in"].ap()[mt * 128:
                                                         (mt + 1) * 128, :],
                                    in_=st)
                            nc.gpsimd.collective_compute(
                                "ReduceScatter", mybir.AluOpType.add,
                                ins=[cc["rs_in"].ap()[qc * 512:
                                                      (qc + 1) * 512, :]],
                                outs=[cc["rs_out"].ap()[qc * 128:
                                                        (qc + 1) * 128, :]],
                                replica_groups=GROUPS)
                    pending.append((idx, wo_rs))
                drain(idx - 1)
            drain(len(units))

        # =================== post-RS: x1 = LN(RS + res1b) =================
        def transposeT(x_tiles, out_tag):
            """4 [128, D] f32 token-major -> 8 [128, CH] bf16 feature-major."""
            outs = [qpool.tile([128, CH], BF16, tag="qt",
                               name=f"{out_tag}{i}") for i in range(DT)]
            for mt in range(NMT):
                for ft in range(DT):
                    pst = ps_m.tile([128, 128], F32, tag="ps_m", name="tps")
                    nc.tensor.transpose(
                        pst, x_tiles[mt][:, ft * 128:(ft + 1) * 128], ident)
                    nc.vector.tensor_copy(
                        outs[ft][:, mt * 128:(mt + 1) * 128], pst)
            return outs

        with nc.named_scope("sa_ln"):
            g1 = bcast_row(d_gbt["sa_g"], "gt", "g1")
            bt1 = bcast_row(d_gbt["sa_bt"], "gt", "bt1")
            x1 = [None] * NMT
            x1T = [qpool.tile([128, CH], BF16, tag="qt", name=f"x1T{i}")
                   for i in range(DT)]
            for mt in (3, 2, 1, 0):   # RS completion order
                rst = scrp.tile([128, D], F8E3, tag="rst", bufs=2, name="rst")
                nc.sync.dma_start(
                    out=rst, in_=cc["rs_out"].ap()[mt * 128:(mt + 1) * 128, :])
                r1t = scrp.tile([128, D], BF16, tag="r1t", bufs=2, name="r1t")
                nc.sync.dma_start(
                    out=r1t, in_=d_res1b.ap()[mt * 128:(mt + 1) * 128, :])
                pre = resp.tile([128, D], F32, tag="persist", name=f"x1_{mt}")
                nc.vector.tensor_add(pre, rst, r1t)
                layer_norm(pre, g1, bt1, pre)
                x1[mt] = pre
                for ft in range(DT):
                    pst = ps_m.tile([128, 128], F32, tag="ps_m", name="tps")
                    nc.tensor.transpose(
                        pst, pre[:, ft * 128:(ft + 1) * 128], ident)
                    nc.vector.tensor_copy(
                        x1T[ft][:, mt * 128:(mt + 1) * 128], pst)

        # =================== CA attention =================================
        with ExitStack() as ca_ctx:
            kvp = ca_ctx.enter_context(tc.tile_pool(name="kvp", bufs=3))
            vpp = ca_ctx.enter_context(tc.tile_pool(name="vpp", bufs=1))
            ppool2 = ca_ctx.enter_context(tc.tile_pool(name="ppool2", bufs=4))
            zpool2 = ca_ctx.enter_context(tc.tile_pool(name="zpool2", bufs=2))
            attp = ca_ctx.enter_context(tc.tile_pool(name="attp", bufs=8))
            zsm2 = ca_ctx.enter_context(tc.tile_pool(name="zsm2", bufs=2))

            with nc.named_scope("ca_q"):
                wqc = load_w8(d_w["ca_wq"])
                QTca = []
                for m in range(DT):
                    ps = ps_m.tile([128, CH], F32, tag="ps_m", name="cqps")
                    for k in range(DT):
                        nc.tensor.matmul(ps, wqc[k][:, m * 128:(m + 1) * 128],
                                         x1T[k], start=(k == 0),
                                         stop=(k == DT - 1))
                    o = qpool.tile([128, CH], BF16, tag="qt", name=f"qca{m}")
                    nc.scalar.activation(out=o, in_=ps, func=AF.Identity,
                                         bias=bq_ca[:, m:m + 1], scale=1.0)
                    QTca.append(o)

            aun = []
            vcur = None

            def ca_z_finisher(zacc_d, at):
                def fin():
                    zs = zsm2.tile([1, 2 * CH], F32, tag="zs", bufs=2,
                                   name="czs")
                    for h2 in range(2):
                        zf = ps_m.tile([1, CH], F32, tag="ps_m", name="czf")
                        nc.tensor.matmul(zf, onescol,
                                         zacc_d[:, h2 * CH:(h2 + 1) * CH],
                                         start=True, stop=True)
                        nc.vector.tensor_copy(zs[:, h2 * CH:(h2 + 1) * CH],
                                              zf)
                    przU = ps_m.tile([128, CH], F32, tag="ps_m", name="cprz")
                    nc.tensor.matmul(przU[0:64, :], onesrow[0:1, :],
                                     zs[:, 0:CH], start=True, stop=True,
                                     tile_position=(0, 0))
                    nc.tensor.matmul(przU[64:128, :], onesrow[0:1, :],
                                     zs[:, CH:2 * CH], start=True, stop=True,
                                     tile_position=(0, 64))
                    rec = zsm2.tile([128, CH], F32, tag="rec", bufs=2,
                                    name="crec")
                    nc.vector.reciprocal_approx_fast(out=rec, in_=przU)
                    nc.vector.tensor_mul(at, at, rec)
                return fin

            ca_pending = []
            for hp in range(H // 2):
                with nc.named_scope(f"ca_pair{hp}"):
                    ktp = kvp.tile([128, 4, CH], F8E3, tag="ktp", name="ktp")
                    nc.sync.dma_start(
                        out=ktp,
                        in_=bass.AP(tensor=cc["kt_out"].ap().tensor,
                                    offset=128 * hp * CH,
                                    ap=[[CH, 128], [D * CH, 4], [1, CH]]))
                    ktb = ktp.rearrange("p a q -> p (a q)")
                    if hp % 2 == 0:
                        vte = vpp.tile([128, NKT, 256], F8E3, tag="vpp",
                                       bufs=1, name="vpp")
                        nc.sync.dma_start(
                            out=vte,
                            in_=bass.AP(tensor=cc["v_out"].ap().tensor,
                                        offset=(hp // 2) * 256,
                                        ap=[[D, 128], [128 * D, NKT],
                                            [1, 256]]))
                        vcur = vte
                    voff = (hp % 2) * 128

                    qa = QTca[hp][0:64, :]
                    qb = QTca[hp][64:128, :]
                    pav = ps_av.tile([128, CH], F32, tag="ps_av", bufs=2,
                                     name="cpav")
                    zacc_d = zpool2.tile([128, 2 * CH], BF16, tag="zd",
                                         bufs=2, name="czd")
                    for kt2 in range(NKT // 2):
                        pt2 = ppool2.tile([128, 2, 2 * CH], BF16, tag="pt",
                                          name="cpt")
                        for sub in range(2):
                            kt = 2 * kt2 + sub
                            pss = ps_s.tile([128, 2 * CH], F32, tag="ps_s",
                                            name="cpss")
                            ksl = ktb[:, kt * 128:(kt + 1) * 128]
                            nc.tensor.matmul(pss[:, 0:CH], ksl[0:64, :], qa,
                                             start=True, stop=True)
                            nc.tensor.matmul(pss[:, CH:2 * CH],
                                             ksl[64:128, :], qb,
                                             start=True, stop=True)
                            nc.scalar.activation(
                                out=pt2[:, sub, :], in_=pss, func=AF.Exp,
                                bias=zerot, scale=1.0 / np.sqrt(DH))
                        if kt2 == 0:
                            nc.vector.tensor_copy(zacc_d, pt2[:, 0, :])
                        else:
                            nc.vector.tensor_add(zacc_d, zacc_d, pt2[:, 0, :])
                        nc.vector.tensor_add(zacc_d, zacc_d, pt2[:, 1, :])
                        for sub in range(2):
                            kt = 2 * kt2 + sub
                            vsl = vcur[:, kt, voff:voff + 128]
                            nc.tensor.matmul(pav[0:64, :], vsl[:, 0:64],
                                             pt2[:, sub, 0:CH],
                                             start=(kt == 0),
                                             stop=(kt == NKT - 1))
                            nc.tensor.matmul(pav[64:128, :], vsl[:, 64:128],
                                             pt2[:, sub, CH:2 * CH],
                                             start=(kt == 0),
                                             stop=(kt == NKT - 1))
                    at = attp.tile([128, CH], BF16, tag="aun", name=f"aun{hp}")
                    nc.vector.tensor_copy(at, pav)
                    aun.append(at)
                    ca_pending.append(ca_z_finisher(zacc_d, at))
                    if len(ca_pending) > 1:
                        ca_pending.pop(0)()
            for fin in ca_pending:
                fin()

            # ---- CA WO + residual(x1) + cvec + LN -> y1, y1T ----
            with nc.named_scope("ca_wo_ln"):
                cvec_t = bcast_row(d_cvec, "vec", "cvec")
                g2 = bcast_row(d_gbt["ca_g"], "gt", "g2")
                bt2 = bcast_row(d_gbt["ca_bt"], "gt", "bt2")
                wo = load_w8(d_w["ca_wo"])
                y1 = []
                for mt in range(NMT):
                    pre = resp.tile([128, D], F32, tag="persist",
                                    name=f"y1_{mt}")
                    for n in range(2):
                        ps = ps_m.tile([128, 512], F32, tag="ps_m",
                                       name="cwops")
                        for k in range(DT):
                            nc.tensor.matmul(
                                ps, aun[k][:, mt * 128:(mt + 1) * 128],
                                wo[k][:, n * 512:(n + 1) * 512],
                                start=(k == 0), stop=(k == DT - 1))
                        nc.vector.tensor_add(pre[:, n * 512:(n + 1) * 512],
                                             ps,
                                             x1[mt][:, n * 512:(n + 1) * 512])
                    nc.vector.tensor_add(pre, pre, cvec_t)
                    layer_norm(pre, g2, bt2, pre)
                    y1.append(pre)
                y1T = transposeT(y1, "y1T")

        # =================== FFN ==========================================
        with ExitStack() as ffn_ctx:
            hpool = ffn_ctx.enter_context(tc.tile_pool(name="hpool", bufs=32))
            w1pool = ffn_ctx.enter_context(tc.tile_pool(name="w1pool", bufs=8))
            with nc.named_scope("ffn1"):
                w1 = []
                for k in range(DT):
                    t = w1pool.tile([128, F], BF16, tag="w1", name=f"w1_{k}")
                    nc.sync.dma_start(out=t,
                                      in_=d_w1.ap()[k * 128:(k + 1) * 128, :])
                    w1.append(t)
                hT = []
                for m in range(NFT):
                    ps = ps_m.tile([128, CH], F32, tag="ps_m", name="f1ps")
                    for k in range(DT):
                        nc.tensor.matmul(ps, w1[k][:, m * 128:(m + 1) * 128],
                                         y1T[k], start=(k == 0),
                                         stop=(k == DT - 1))
                    h = hpool.tile([128, CH], BF16, tag="h", name=f"h{m}")
                    nc.scalar.activation(out=h, in_=ps, func=AF.Relu,
                                         bias=b1c[:, m:m + 1], scale=1.0)
                    hT.append(h)
            with nc.named_scope("ffn2"):
                b2v_t = bcast_row(d_b2v, "vec", "b2v")
                h2 = [resp.tile([128, D], F32, tag="persist", name=f"h2_{i}")
                      for i in range(NMT)]
                for n in range(2):
                    pss = [ps_s.tile([128, 2 * CH], F32, tag="ps_s",
                                     name=f"f2ps{n}_{i}") for i in range(2)]
                    for kb in range(4):
                        w2b = w1pool.tile([128, 8, 512], BF16, tag="w1",
                                          name=f"w2b{kb}")
                        nc.sync.dma_start(
                            out=w2b,
                            in_=bass.AP(tensor=d_w2.ap().tensor,
                                        offset=kb * 8 * 128 * D + n * 512,
                                        ap=[[D, 128], [128 * D, 8], [1, 512]]))
                        for ks in range(8):
                            k = kb * 8 + ks
                            for mt in range(NMT):
                                nc.tensor.matmul(
                                    pss[mt // 2][:, (mt % 2) * CH:
                                                 (mt % 2 + 1) * CH],
                                    hT[k][:, mt * 128:(mt + 1) * 128],
                                    w2b[:, ks, :],
                                    start=(k == 0), stop=(k == NFT - 1))
                    for mt in range(NMT):
                        nc.vector.tensor_add(
                            h2[mt][:, n * 512:(n + 1) * 512],
                            pss[mt // 2][:, (mt % 2) * CH:(mt % 2 + 1) * CH],
                            y1[mt][:, n * 512:(n + 1) * 512])
            with nc.named_scope("ln3_out"):
                g3 = bcast_row(d_gbt["f_g"], "gt", "g3")
                bt3 = bcast_row(d_gbt["f_bt"], "gt", "bt3")
                for mt in range(NMT):
                    nc.vector.tensor_add(h2[mt], h2[mt], b2v_t)
                    layer_norm(h2[mt], g3, bt3, h2[mt])
                    nc.sync.dma_start(out=d_out.ap()[mt * 128:(mt + 1) * 128, :],
                                      in_=h2[mt])

    nc.compile()
    return nc


def _bf(a):
    return np.ascontiguousarray(a, dtype=np.float32).astype(ml_dtypes.bfloat16)


def kernel(**inputs):
    global _CACHED
    if _CACHED is None:
        _CACHED = build()
    nc = _CACHED

    f = {k: np.asarray(v, dtype=np.float32) for k, v in inputs.items()}
    dec, enc = f["decoder_input"], f["encoder_output"]
    cvec = (f["ca_bv"] @ f["ca_wo"] + f["ca_bo"]).astype(np.float32)
    r1vec = (f["sa_bv"] @ f["sa_wo"] + f["sa_bo"]).astype(np.float32)

    shared = {n: _bf(f[n]) for n in
              ["ca_wq", "ca_wk", "ca_wv", "ca_wo", "f_w1", "f_w2"]}
    shared.update({n: f[n] for n in ["ca_bq", "ca_bk", "f_b1"]})
    shared["cvec"] = _bf(cvec)
    shared["b2v"] = _bf(f["f_b2"])
    for n in ["sa_g", "sa_bt", "ca_g", "ca_bt", "f_g", "f_bt"]:
        shared[n] = _bf(f[n])

    in_maps = []
    rows_of = {}
    for c in range(8):
        b, j = c // 4, c % 4
        rows = np.concatenate([np.arange(512 * qc + 128 * j,
                                         512 * qc + 128 * j + 128)
                               for qc in range(4)])
        rows_of[c] = rows
        hs = slice(j * 256, (j + 1) * 256)
        m = {
            "xT": _bf(dec[b].T),
            "eTq": _bf(enc[b, rows, :].T),
            "res1b": _bf(dec[b, rows, :] + r1vec),
            "sa_wq_h": _bf(f["sa_wq"][:, hs]),
            "sa_wk_h": _bf(f["sa_wk"][:, hs]),
            "sa_wv_h": _bf(f["sa_wv"][:, hs]),
            "sa_wo_h": _bf(f["sa_wo"][hs, :]),
            "sa_bq_h": np.ascontiguousarray(f["sa_bq"][hs], dtype=np.float32),
            "sa_bk_h": np.ascontiguousarray(f["sa_bk"][hs], dtype=np.float32),
        }
        m.update(shared)
        in_maps.append(m)

    global LAST_RES
    res = bass_utils.run_bass_kernel_spmd(nc, in_maps, core_ids=list(range(8)))
    LAST_RES = res
    out = np.empty((B, S, D), dtype=np.float32)
    for c in range(8):
        b = c // 4
        out[b, rows_of[c], :] = res.results[c]["out"]
    return out


# revision 11
# speedup vs baseline: 1.1724x; 1.0584x over previous
"""Transformer decoder block (self-attn + cross-attn + FFN, post-LN) on 8
Trainium2 NeuronCores.

v3: head-sharded causal self-attention + token-sharded cross-attn/FFN.

8 cores = 2 batches x 4 ranks. Rank j of a batch group:
  - SA: computes heads [4j, 4j+4) for ALL 2048 tokens. K/V/Q projected
    locally from the full decoder input (no collective before attention).
    Causal structure is uniform across cores: per 512-query chunk qc only
    key tiles kt < 4(qc+1) are computed (62.5% of the full score work) and
    only the 4 diagonal tiles are masked. WO partials [2048, 1024] are
    staged in fp8e3 and summed across the 4 ranks with two column-split
    ReduceScatters; rank j receives its own 512-token slice.
  - CA: token-sharded as v2 — each rank projects K/V for its 512 encoder
    tokens, one fused AllGather per tensor (fp8e3), attention for its 512
    queries over all 2048 keys. The CA AllGathers are issued early and fly
    under the SA compute.
  - FFN + all residual/LN paths: token-sharded (512 tokens per rank).

v3 micro-optimizations vs v2:
  - CA score/AV matmuls read the fp8 AllGather buffers directly as the
    stationary operand (mixed fp8xbf16 matmul) — the fp8->bf16 DVE casts
    are gone.
  - K/V transport in fp8e3 (e3m4) instead of e4m3: halves the
    quantization error of the collective path.
  - softmax 1/Z via reciprocal_approx_fast (~5x faster than reciprocal).
  - Z-accumulation split between the DVE and the (otherwise idle) GpSimd
    engine: two partial accumulators, merged by the partition-sum matmul.
  - residual 1 (decoder input + bv@wo+bo) precomputed host-side and DMAd
    in [token, feature] layout directly (no PE transposes to rebuild it).
All matmuls bf16 (or fp8e3 stationary) with fp32 PSUM accumulation.
"""

from contextlib import ExitStack

import numpy as np
import ml_dtypes

import concourse.bass as bass
import concourse.bacc as bacc
import concourse.mybir as mybir
import concourse.tile as tile
from concourse import bass_utils
from concourse.masks import make_identity

BF16 = mybir.dt.bfloat16
F8E3 = mybir.dt.float8e3
F32 = mybir.dt.float32
AF = mybir.ActivationFunctionType
OP = mybir.AluOpType

B, S, D, H, F = 2, 2048, 1024, 16, 4096
DH = 64
EPS = 1e-5
CH = 512          # output tokens per core
DT = D // 128     # 8 feature tiles
NKT = S // 128    # 16 key tiles
NMT = CH // 128   # 4 token tiles per core (output)
NFT = F // 128    # 32 FFN hidden tiles
NST = S // 128    # 16 token tiles (full sequence)

_CACHED = None


def build():
    nc = bacc.Bacc("TRN2", target_bir_lowering=False, debug=False,
                   enable_asserts=False, num_devices=8)

    # ---- per-core DRAM I/O ----
    d_xT = nc.dram_tensor("xT", [D, S], BF16, kind="ExternalInput")
    d_eTq = nc.dram_tensor("eTq", [D, CH], BF16, kind="ExternalInput")
    d_res1b = nc.dram_tensor("res1b", [CH, D], BF16, kind="ExternalInput")
    d_wqh = nc.dram_tensor("sa_wq_h", [D, 256], BF16, kind="ExternalInput")
    d_wkh = nc.dram_tensor("sa_wk_h", [D, 256], BF16, kind="ExternalInput")
    d_wvh = nc.dram_tensor("sa_wv_h", [D, 256], BF16, kind="ExternalInput")
    d_woh = nc.dram_tensor("sa_wo_h", [256, D], BF16, kind="ExternalInput")
    d_bqh = nc.dram_tensor("sa_bq_h", [256], F32, kind="ExternalInput")
    d_bkh = nc.dram_tensor("sa_bk_h", [256], F32, kind="ExternalInput")
    cnames = ["ca_wq", "ca_wk", "ca_wv", "ca_wo"]
    d_w = {n: nc.dram_tensor(n, [D, D], BF16, kind="ExternalInput")
           for n in cnames}
    d_w1 = nc.dram_tensor("f_w1", [D, F], BF16, kind="ExternalInput")
    d_w2 = nc.dram_tensor("f_w2", [F, D], BF16, kind="ExternalInput")
    d_bq_ca = nc.dram_tensor("ca_bq", [D], F32, kind="ExternalInput")
    d_bk_ca = nc.dram_tensor("ca_bk", [D], F32, kind="ExternalInput")
    d_b1 = nc.dram_tensor("f_b1", [F], F32, kind="ExternalInput")
    d_cvec = nc.dram_tensor("cvec", [D], BF16, kind="ExternalInput")
    d_b2v = nc.dram_tensor("b2v", [D], BF16, kind="ExternalInput")
    d_gbt = {n: nc.dram_tensor(n, [D], BF16, kind="ExternalInput")
             for n in ["sa_g", "sa_bt", "ca_g", "ca_bt", "f_g", "f_bt"]}
    d_out = nc.dram_tensor("out", [CH, D], F32, kind="ExternalOutput")

    cc = {
        "kt_in": nc.dram_tensor("cc_ca_kt_in", [D, CH], F8E3, kind="Internal"),
        "kt_out": nc.dram_tensor("cc_ca_kt_out", [4 * D, CH], F8E3,
                                 kind="Internal"),
        "v_in": nc.dram_tensor("cc_ca_v_in", [CH, D], F8E3, kind="Internal"),
        "v_out": nc.dram_tensor("cc_ca_v_out", [S, D], F8E3, kind="Internal"),
        "rs_in": nc.dram_tensor("cc_rs_in", [S, D], F8E3, kind="Internal"),
        "rs_out": nc.dram_tensor("cc_rs_out", [CH, D], F8E3, kind="Internal"),
    }
    GROUPS = [[0, 1, 2, 3], [4, 5, 6, 7]]

    with tile.TileContext(nc) as tc, ExitStack() as ctx:
        const = ctx.enter_context(tc.tile_pool(name="const", bufs=1))
        wpool = ctx.enter_context(tc.tile_pool(name="wpool", bufs=8))
        qpool = ctx.enter_context(tc.tile_pool(name="qpool", bufs=16))
        resp = ctx.enter_context(tc.tile_pool(name="resp", bufs=8))
        scrp = ctx.enter_context(tc.tile_pool(name="scrp", bufs=2))
        ps_s = ctx.enter_context(tc.tile_pool(name="ps_s", bufs=2, space="PSUM"))
        ps_av = ctx.enter_context(tc.tile_pool(name="ps_av", bufs=2, space="PSUM"))
        ps_m = ctx.enter_context(tc.tile_pool(name="ps_m", bufs=2, space="PSUM"))

        ident = const.tile([128, 128], F32, tag="ident")
        make_identity(nc, ident)
        identb = const.tile([128, 128], BF16, tag="identb")
        nc.vector.tensor_copy(identb, ident)
        onescol = const.tile([128, 1], BF16, tag="onescol")
        nc.vector.memset(onescol, 1.0)
        onesrow = const.tile([1, 64], F32, tag="onesrow")
        nc.vector.memset(onesrow, 1.0)
        epst = const.tile([128, 1], F32, tag="epst")
        nc.vector.memset(epst, EPS)
        zerot = const.tile([128, 1], F32, tag="zerot")
        nc.vector.memset(zerot, 0.0)

        def bias_cols(dram, ntiles, name):
            t = const.tile([128, ntiles], F32, tag=name, name=name)
            src = bass.AP(tensor=dram.ap().tensor, offset=0,
                          ap=[[1, 128], [128, ntiles]])
            nc.sync.dma_start(out=t, in_=src)
            return t

        def bcast_row(dram, tag, name):
            t = const.tile([128, D], BF16, tag=tag, bufs=2, name=name)
            src = bass.AP(tensor=dram.ap().tensor, offset=0, ap=[[0, 128], [1, D]])
            nc.sync.dma_start(out=t, in_=src)
            return t

        bq_h = bias_cols(d_bqh, 2, "bqh")
        bk_h = bias_cols(d_bkh, 2, "bkh")
        bq_ca = bias_cols(d_bq_ca, DT, "bqca")
        bk_ca = bias_cols(d_bk_ca, DT, "bkca")
        b1c = bias_cols(d_b1, NFT, "b1c")

        def layer_norm(src, g_t, bt_t, out):
            """[128, D] f32 LN along free dim; out may alias src."""
            stats = scrp.tile([128, 2, 6], F32, tag="lnstat", name="lnstat")
            for s in range(2):
                nc.vector.bn_stats(out=stats[:, s, :],
                                   in_=src[:, s * 512:(s + 1) * 512])
            mv = scrp.tile([128, 2], F32, tag="lnmv", name="lnmv")
            nc.vector.bn_aggr(out=mv, in_=stats)
            rstd = scrp.tile([128, 1], F32, tag="lnrstd", name="lnrstd")
            nc.scalar.activation(out=rstd, in_=mv[:, 1:2], func=AF.Sqrt,
                                 bias=epst, scale=1.0)
            nc.vector.reciprocal(out=rstd, in_=rstd)
            cent = scrp.tile([128, D], F32, tag="scr", name="cent")
            nc.vector.scalar_tensor_tensor(out=cent, in0=src, scalar=mv[:, 0:1],
                                           in1=g_t, op0=OP.subtract, op1=OP.mult)
            nc.vector.scalar_tensor_tensor(out=out, in0=cent, scalar=rstd,
                                           in1=bt_t, op0=OP.mult, op1=OP.add)

        def load_w8(wd, ncols=D):
            ws = []
            for k in range(DT):
                t = wpool.tile([128, ncols], BF16, tag="w", name=f"w_{k}")
                nc.sync.dma_start(out=t, in_=wd.ap()[k * 128:(k + 1) * 128, :])
                ws.append(t)
            return ws

        # =================== SA input prefetch + pools ====================
        sa_ctx = ExitStack()
        big = sa_ctx.enter_context(tc.tile_pool(name="big", bufs=2))
        maskp = sa_ctx.enter_context(tc.tile_pool(name="maskp", bufs=1))
        ppool = sa_ctx.enter_context(tc.tile_pool(name="ppool", bufs=3))
        zpool = sa_ctx.enter_context(tc.tile_pool(name="zpool", bufs=2))
        zsm = sa_ctx.enter_context(tc.tile_pool(name="zsm", bufs=2))
        proj_ctx = ExitStack()
        projp = proj_ctx.enter_context(tc.tile_pool(name="projp", bufs=8))
        xt = []
        for k in range(DT):
            t = projp.tile([128, S], BF16, tag="xt", bufs=8, name=f"xt{k}")
            nc.sync.dma_start(out=t, in_=d_xT.ap()[k * 128:(k + 1) * 128, :])
            xt.append(t)

        def load_wh(wd, tag):
            ws = []
            for k in range(DT):
                t = projp.tile([128, 256], BF16, tag=tag, bufs=8,
                               name=f"{tag}{k}")
                nc.sync.dma_start(out=t, in_=wd.ap()[k * 128:(k + 1) * 128, :])
                ws.append(t)
            return ws

        wqh = load_wh(d_wqh, "wqh")
        wkh = load_wh(d_wkh, "wkh")
        wvh = load_wh(d_wvh, "wvh")

        # =================== CA K/V local + AllGathers (first) ============
        with nc.named_scope("ca_kvlocal"):
            eq = []
            for k in range(DT):
                t = qpool.tile([128, CH], BF16, tag="qt", name=f"eq{k}")
                nc.sync.dma_start(out=t, in_=d_eTq.ap()[k * 128:(k + 1) * 128, :])
                eq.append(t)
            wk = load_w8(d_w["ca_wk"])
            for m in range(DT):
                ps = ps_m.tile([128, CH], F32, tag="ps_m", name="lkps")
                for k in range(DT):
                    nc.tensor.matmul(ps, wk[k][:, m * 128:(m + 1) * 128],
                                     eq[k], start=(k == 0), stop=(k == DT - 1))
                st = scrp.tile([128, CH], F8E3, tag="stage", bufs=4, name="ktst")
                nc.scalar.activation(out=st, in_=ps, func=AF.Identity,
                                     bias=bk_ca[:, m:m + 1], scale=1.0)
                nc.sync.dma_start(
                    out=cc["kt_in"].ap()[m * 128:(m + 1) * 128, :], in_=st)
            nc.gpsimd.collective_compute(
                "AllGather", mybir.AluOpType.bypass,
                ins=[cc["kt_in"].ap()], outs=[cc["kt_out"].ap()],
                replica_groups=GROUPS)
            wv = load_w8(d_w["ca_wv"])
            for tt in range(NMT):
                for n in range(2):
                    ps = ps_m.tile([128, CH], F32, tag="ps_m", name="lvps")
                    for k in range(DT):
                        nc.tensor.matmul(
                            ps, eq[k][:, tt * 128:(tt + 1) * 128],
                            wv[k][:, n * 512:(n + 1) * 512],
                            start=(k == 0), stop=(k == DT - 1))
                    st = scrp.tile([128, CH], F8E3, tag="stage", bufs=4,
                                   name="vst")
                    nc.scalar.activation(out=st, in_=ps, func=AF.Copy)
                    nc.sync.dma_start(
                        out=cc["v_in"].ap()[tt * 128:(tt + 1) * 128,
                                            n * 512:(n + 1) * 512],
                        in_=st)
            nc.gpsimd.collective_compute(
                "AllGather", mybir.AluOpType.bypass,
                ins=[cc["v_in"].ap()], outs=[cc["v_out"].ap()],
                replica_groups=GROUPS)

        # =================== SA local projections =========================
        if True:
            # K^T and Q^T: [128 (2 heads x 64 dh), 2048 tokens] per pair
            KT = [big.tile([128, S], BF16, tag="KT", bufs=2, name=f"KT{g}")
                  for g in range(2)]
            QT = [big.tile([128, S], BF16, tag="QT", bufs=2, name=f"QT{g}")
                  for g in range(2)]
            # V: [128 tokens per tile, 16 tiles, 256 dh]  (bias folded)
            vt = big.tile([128, NST, 256], F8E3, tag="vt", bufs=1, name="vt")

            with nc.named_scope("sa_proj"):
                for g in range(2):
                    for tc4 in range(4):
                        ps = ps_m.tile([128, CH], F32, tag="ps_m", name="kps")
                        for k in range(DT):
                            nc.tensor.matmul(
                                ps, wkh[k][:, g * 128:(g + 1) * 128],
                                xt[k][:, tc4 * 512:(tc4 + 1) * 512],
                                start=(k == 0), stop=(k == DT - 1))
                        nc.scalar.activation(
                            out=KT[g][:, tc4 * 512:(tc4 + 1) * 512], in_=ps,
                            func=AF.Identity, bias=bk_h[:, g:g + 1], scale=1.0)
                for g in range(2):
                    for tc4 in range(4):
                        ps = ps_m.tile([128, CH], F32, tag="ps_m", name="qps")
                        for k in range(DT):
                            nc.tensor.matmul(
                                ps, wqh[k][:, g * 128:(g + 1) * 128],
                                xt[k][:, tc4 * 512:(tc4 + 1) * 512],
                                start=(k == 0), stop=(k == DT - 1))
                        nc.scalar.activation(
                            out=QT[g][:, tc4 * 512:(tc4 + 1) * 512], in_=ps,
                            func=AF.Identity, bias=bq_h[:, g:g + 1], scale=1.0)
                for tt in range(NST):
                    ps = ps_m.tile([128, 256], F32, tag="ps_m", name="vps")
                    for k in range(DT):
                        nc.tensor.matmul(
                            ps, xt[k][:, tt * 128:(tt + 1) * 128], wvh[k],
                            start=(k == 0), stop=(k == DT - 1))
                    nc.scalar.activation(out=vt[:, tt, :], in_=ps,
                                         func=AF.Copy)

            proj_ctx.close()
            # ---- causal mask for the 4 diagonal tiles (same on all cores):
            # amask[k, dkt, q] = -240 if (128*dkt + k > q) else 0
            qmk = scrp.tile([128, CH], F32, tag="qmk", bufs=1, name="qmk")
            nc.gpsimd.iota(qmk, pattern=[[1, CH]], base=0,
                           channel_multiplier=-1,
                           allow_small_or_imprecise_dtypes=True)
            amask = maskp.tile([128, 4, CH], BF16, tag="mask", name="amask")
            for dkt in range(4):
                nc.vector.tensor_scalar(out=amask[:, dkt, :], in0=qmk,
                                        scalar1=float(128 * dkt),
                                        scalar2=-240.0, op0=OP.is_lt,
                                        op1=OP.mult)

            # =================== SA attention pairs =======================
            # qc-outer: after both pair-groups of a 512-token chunk finish,
            # its WO partial is computed and chunk-ReduceScatter qc fires --
            # RS 0..2 fly under the remaining pairs, only RS 3 is exposed.
            attnT = [big.tile([128, S], BF16, tag="atn", bufs=2,
                              name=f"atn{g}") for g in range(2)]
            woh = [big.tile([128, D], BF16, tag="woh", bufs=2,
                            name=f"woh{g}") for g in range(2)]
            for g in range(2):
                nc.sync.dma_start(
                    out=woh[g], in_=d_woh.ap()[g * 128:(g + 1) * 128, :])
            GPN = {0: 0, 1: 0, 2: 1, 3: 1}   # trailing kt2 iters on GpSimd

            def z_finisher(zacc_d, zacc_g, ngp, at):
                def fin():
                    zs = zsm.tile([1, 2 * CH], F32, tag="zs", bufs=2,
                                  name="zs")
                    for h2 in range(2):
                        zf = ps_m.tile([1, CH], F32, tag="ps_m", name="zf")
                        nc.tensor.matmul(zf, onescol,
                                         zacc_d[:, h2 * CH:(h2 + 1) * CH],
                                         start=True, stop=(ngp == 0))
                        if ngp:
                            nc.tensor.matmul(
                                zf, onescol,
                                zacc_g[:, h2 * CH:(h2 + 1) * CH],
                                start=False, stop=True)
                        nc.vector.tensor_copy(zs[:, h2 * CH:(h2 + 1) * CH],
                                              zf)
                    przU = ps_m.tile([128, CH], F32, tag="ps_m", name="przU")
                    nc.tensor.matmul(przU[0:64, :], onesrow[0:1, :],
                                     zs[:, 0:CH], start=True, stop=True,
                                     tile_position=(0, 0))
                    nc.tensor.matmul(przU[64:128, :], onesrow[0:1, :],
                                     zs[:, CH:2 * CH], start=True,
                                     stop=True, tile_position=(0, 64))
                    rec = zsm.tile([128, CH], F32, tag="rec", bufs=2,
                                   name="rec")
                    nc.vector.reciprocal_approx_fast(out=rec, in_=przU)
                    nc.vector.tensor_mul(at, at, rec)
                return fin

            # finishers run lagged by one pair-unit so the PE queue never
            # waits on the previous pair's Z chain.
            pending = []   # (unit_idx, closure)

            def drain(upto):
                while pending and pending[0][0] <= upto:
                    pending.pop(0)[1]()

            units = [(qc, g) for qc in (3, 2, 1, 0) for g in (0, 1)]
            for idx, (qc, g) in enumerate(units):
                with nc.named_scope(f"sa_g{g}q{qc}"):
                    nkt = 4 * qc + 4
                    nkt2 = nkt // 2
                    ngp = GPN[qc]
                    ndv = nkt2 - ngp
                    qa = QT[g][0:64, qc * 512:(qc + 1) * 512]
                    qb = QT[g][64:128, qc * 512:(qc + 1) * 512]
                    pav = ps_av.tile([128, CH], F32, tag="ps_av", bufs=2,
                                     name="pav")
                    zacc_d = zpool.tile([128, 2 * CH], BF16, tag="zd",
                                        bufs=2, name="zacc_d")
                    zacc_g = zpool.tile([128, 2 * CH], BF16, tag="zg",
                                        bufs=2, name="zacc_g")
                    for kt2 in range(nkt2):
                        if kt2 == 1:
                            drain(idx - 1)
                        pt2 = ppool.tile([128, 2, 2 * CH], BF16, tag="pt",
                                         name="pt")
                        for sub in range(2):
                            kt = 2 * kt2 + sub
                            diag = kt >= 4 * qc
                            pss = ps_s.tile([128, 2 * CH], F32, tag="ps_s",
                                            name="pss")
                            ksl = KT[g][:, kt * 128:(kt + 1) * 128]
                            nc.tensor.matmul(pss[:, 0:CH], ksl[0:64, :],
                                             qa, start=True,
                                             stop=not diag)
                            nc.tensor.matmul(pss[:, CH:2 * CH],
                                             ksl[64:128, :], qb,
                                             start=True, stop=True)
                            if diag:
                                dkt = kt - 4 * qc
                                nc.tensor.matmul(pss[:, 0:CH], identb,
                                                 amask[:, dkt, :],
                                                 start=False, stop=True,
                                                 skip_group_check=True)
                            nc.scalar.activation(
                                out=pt2[:, sub, :], in_=pss, func=AF.Exp,
                                bias=zerot, scale=1.0 / np.sqrt(DH))
                            if diag:
                                dkt = kt - 4 * qc
                                nc.vector.scalar_tensor_tensor(
                                    out=pt2[:, sub, CH:2 * CH],
                                    in0=amask[:, dkt, :],
                                    scalar=-1.0, op0=OP.is_ge,
                                    in1=pt2[:, sub, CH:2 * CH],
                                    op1=OP.mult)
                        if kt2 < ndv:
                            eng, zt, first = nc.vector, zacc_d, kt2 == 0
                        else:
                            eng, zt, first = nc.gpsimd, zacc_g, kt2 == ndv
                        if first:
                            eng.tensor_copy(zt, pt2[:, 0, :])
                        else:
                            eng.tensor_add(zt, zt, pt2[:, 0, :])
                        eng.tensor_add(zt, zt, pt2[:, 1, :])
                        for sub in range(2):
                            kt = 2 * kt2 + sub
                            vsl = vt[:, kt, g * 128:(g + 1) * 128]
                            nc.tensor.matmul(pav[0:64, :], vsl[:, 0:64],
                                             pt2[:, sub, 0:CH],
                                             start=(kt == 0),
                                             stop=(kt == nkt - 1))
                            nc.tensor.matmul(pav[64:128, :],
                                             vsl[:, 64:128],
                                             pt2[:, sub, CH:2 * CH],
                                             start=(kt == 0),
                                             stop=(kt == nkt - 1))
                    at = attnT[g][:, qc * 512:(qc + 1) * 512]
                    nc.vector.tensor_copy(at, pav)
                    drain(idx - 1)
                    pending.append((idx, z_finisher(zacc_d, zacc_g, ngp, at)))
                if g == 1:
                    def wo_rs(qc=qc):
                        with nc.named_scope(f"sa_wo{qc}"):
                            for mt in range(4 * qc, 4 * qc + 4):
                                st = scrp.tile([128, D], F8E3, tag="stage2",
                                               bufs=4, name="rsst")
                                for n in range(2):
                                    ps = ps_m.tile([128, CH], F32, tag="ps_m",
                                                   name="wops")
                                    for gg in range(2):
                                        nc.tensor.matmul(
                                            ps,
                                            attnT[gg][:, mt * 128:
                                                      (mt + 1) * 128],
                                            woh[gg][:, n * 512:(n + 1) * 512],
                                            start=(gg == 0), stop=(gg == 1))
                                    if (mt + n) % 2 == 0:
                                        nc.scalar.activation(
                                            out=st[:, n * 512:(n + 1) * 512],
                                            in_=ps, func=AF.Copy)
                                    else:
                                        nc.vector.tensor_copy(
                                            st[:, n * 512:(n + 1) * 512], ps)
                                nc.sync.dma_start(
                                    out=cc["rs_in"].ap()[mt * 128:
                                                         (mt + 1) * 128, :],
                                    in_=st)
                            nc.gpsimd.collective_compute(
                                "ReduceScatter", mybir.AluOpType.add,
                                ins=[cc["rs_in"].ap()[qc * 512:
                                                      (qc + 1) * 512, :]],
                                outs=[cc["rs_out"].ap()[qc * 128:
                                                        (qc + 1) * 128, :]],
                                replica_groups=GROUPS)
                    pending.append((idx, wo_rs))
            drain(len(units))

        sa_ctx.close()

        # =================== post-RS: x1 = LN(RS + res1b) =================
        def transposeT(x_tiles, out_tag):
            """4 [128, D] f32 token-major -> 8 [128, CH] bf16 feature-major."""
            outs = [qpool.tile([128, CH], BF16, tag="qt",
                               name=f"{out_tag}{i}") for i in range(DT)]
            for mt in range(NMT):
                for ft in range(DT):
                    pst = ps_m.tile([128, 128], F32, tag="ps_m", name="tps")
                    nc.tensor.transpose(
                        pst, x_tiles[mt][:, ft * 128:(ft + 1) * 128], ident)
                    nc.vector.tensor_copy(
                        outs[ft][:, mt * 128:(mt + 1) * 128], pst)
            return outs

        with nc.named_scope("sa_ln"):
            g1 = bcast_row(d_gbt["sa_g"], "gt", "g1")
            bt1 = bcast_row(d_gbt["sa_bt"], "gt", "bt1")
            x1 = [None] * NMT
            x1T = [qpool.tile([128, CH], BF16, tag="qt", name=f"x1T{i}")
                   for i in range(DT)]
            for mt in (3, 2, 1, 0):   # RS completion order
                rst = scrp.tile([128, D], F8E3, tag="rst", bufs=2, name="rst")
                nc.sync.dma_start(
                    out=rst, in_=cc["rs_out"].ap()[mt * 128:(mt + 1) * 128, :])
                r1t = scrp.tile([128, D], BF16, tag="r1t", bufs=2, name="r1t")
                nc.sync.dma_start(
                    out=r1t, in_=d_res1b.ap()[mt * 128:(mt + 1) * 128, :])
                pre = resp.tile([128, D], F32, tag="persist", name=f"x1_{mt}")
                nc.vector.tensor_add(pre, rst, r1t)
                layer_norm(pre, g1, bt1, pre)
                x1[mt] = pre
                for ft in range(DT):
                    pst = ps_m.tile([128, 128], F32, tag="ps_m", name="tps")
                    nc.tensor.transpose(
                        pst, pre[:, ft * 128:(ft + 1) * 128], ident)
                    nc.vector.tensor_copy(
                        x1T[ft][:, mt * 128:(mt + 1) * 128], pst)

        # =================== CA attention =================================
        with ExitStack() as ca_ctx:
            kvp = ca_ctx.enter_context(tc.tile_pool(name="kvp", bufs=3))
            vpp = ca_ctx.enter_context(tc.tile_pool(name="vpp", bufs=1))
            ppool2 = ca_ctx.enter_context(tc.tile_pool(name="ppool2", bufs=4))
            zpool2 = ca_ctx.enter_context(tc.tile_pool(name="zpool2", bufs=2))
            attp = ca_ctx.enter_context(tc.tile_pool(name="attp", bufs=8))
            zsm2 = ca_ctx.enter_context(tc.tile_pool(name="zsm2", bufs=2))

            with nc.named_scope("ca_q"):
                wqc = load_w8(d_w["ca_wq"])
                QTca = []
                for m in range(DT):
                    ps = ps_m.tile([128, CH], F32, tag="ps_m", name="cqps")
                    for k in range(DT):
                        nc.tensor.matmul(ps, wqc[k][:, m * 128:(m + 1) * 128],
                                         x1T[k], start=(k == 0),
                                         stop=(k == DT - 1))
                    o = qpool.tile([128, CH], BF16, tag="qt", name=f"qca{m}")
                    nc.scalar.activation(out=o, in_=ps, func=AF.Identity,
                                         bias=bq_ca[:, m:m + 1], scale=1.0)
                    QTca.append(o)

            aun = []
            vcur = None

            def ca_z_finisher(zacc_d, at):
                def fin():
                    zs = zsm2.tile([1, 2 * CH], F32, tag="zs", bufs=2,
                                   name="czs")
                    for h2 in range(2):
                        zf = ps_m.tile([1, CH], F32, tag="ps_m", name="czf")
                        nc.tensor.matmul(zf, onescol,
                                         zacc_d[:, h2 * CH:(h2 + 1) * CH],
                                         start=True, stop=True)
                        nc.vector.tensor_copy(zs[:, h2 * CH:(h2 + 1) * CH],
                                              zf)
                    przU = ps_m.tile([128, CH], F32, tag="ps_m", name="cprz")
                    nc.tensor.matmul(przU[0:64, :], onesrow[0:1, :],
                                     zs[:, 0:CH], start=True, stop=True,
                                     tile_position=(0, 0))
                    nc.tensor.matmul(przU[64:128, :], onesrow[0:1, :],
                                     zs[:, CH:2 * CH], start=True, stop=True,
                                     tile_position=(0, 64))
                    rec = zsm2.tile([128, CH], F32, tag="rec", bufs=2,
                                    name="crec")
                    nc.vector.reciprocal_approx_fast(out=rec, in_=przU)
                    nc.vector.tensor_mul(at, at, rec)
                return fin

            ca_pending = []
            for hp in range(H // 2):
                with nc.named_scope(f"ca_pair{hp}"):
                    ktp = kvp.tile([128, 4, CH], F8E3, tag="ktp", name="ktp")
                    nc.sync.dma_start(
                        out=ktp,
                        in_=bass.AP(tensor=cc["kt_out"].ap().tensor,
                                    offset=128 * hp * CH,
                                    ap=[[CH, 128], [D * CH, 4], [1, CH]]))
                    ktb = ktp.rearrange("p a q -> p (a q)")
                    if hp % 2 == 0:
                        vte = vpp.tile([128, NKT, 256], F8E3, tag="vpp",
                                       bufs=1, name="vpp")
                        nc.sync.dma_start(
                            out=vte,
                            in_=bass.AP(tensor=cc["v_out"].ap().tensor,
                                        offset=(hp // 2) * 256,
                                        ap=[[D, 128], [128 * D, NKT],
                                            [1, 256]]))
                        vcur = vte
                    voff = (hp % 2) * 128

                    qa = QTca[hp][0:64, :]
                    qb = QTca[hp][64:128, :]
                    pav = ps_av.tile([128, CH], F32, tag="ps_av", bufs=2,
                                     name="cpav")
                    zacc_d = zpool2.tile([128, 2 * CH], BF16, tag="zd",
                                         bufs=2, name="czd")
                    for kt2 in range(NKT // 2):
                        if kt2 == 1 and ca_pending:
                            ca_pending.pop(0)()
                        pt2 = ppool2.tile([128, 2, 2 * CH], BF16, tag="cpt",
                                          name="cpt")
                        for sub in range(2):
                            kt = 2 * kt2 + sub
                            pss = ps_s.tile([128, 2 * CH], F32, tag="ps_s",
                                            name="cpss")
                            ksl = ktb[:, kt * 128:(kt + 1) * 128]
                            nc.tensor.matmul(pss[:, 0:CH], ksl[0:64, :], qa,
                                             start=True, stop=True)
                            nc.tensor.matmul(pss[:, CH:2 * CH],
                                             ksl[64:128, :], qb,
                                             start=True, stop=True)
                            nc.scalar.activation(
                                out=pt2[:, sub, :], in_=pss, func=AF.Exp,
                                bias=zerot, scale=1.0 / np.sqrt(DH))
                        if kt2 == 0:
                            nc.vector.tensor_copy(zacc_d, pt2[:, 0, :])
                        else:
                            nc.vector.tensor_add(zacc_d, zacc_d, pt2[:, 0, :])
                        nc.vector.tensor_add(zacc_d, zacc_d, pt2[:, 1, :])
                        for sub in range(2):
                            kt = 2 * kt2 + sub
                            vsl = vcur[:, kt, voff:voff + 128]
                            nc.tensor.matmul(pav[0:64, :], vsl[:, 0:64],
                                             pt2[:, sub, 0:CH],
                                             start=(kt == 0),
                                             stop=(kt == NKT - 1))
                            nc.tensor.matmul(pav[64:128, :], vsl[:, 64:128],
                                             pt2[:, sub, CH:2 * CH],
                                             start=(kt == 0),
                                             stop=(kt == NKT - 1))
                    at = attp.tile([128, CH], BF16, tag="aun", name=f"aun{hp}")
                    nc.vector.tensor_copy(at, pav)
                    aun.append(at)
                    ca_pending.append(ca_z_finisher(zacc_d, at))
            for fin in ca_pending:
                fin()

            # ---- CA WO + residual(x1) + cvec + LN -> y1, y1T ----
            with nc.named_scope("ca_wo_ln"):
                cvec_t = bcast_row(d_cvec, "vec", "cvec")
                g2 = bcast_row(d_gbt["ca_g"], "gt", "g2")
                bt2 = bcast_row(d_gbt["ca_bt"], "gt", "bt2")
                wo = load_w8(d_w["ca_wo"])
                y1 = []
                for mt in range(NMT):
                    pre = resp.tile([128, D], F32, tag="persist",
                                    name=f"y1_{mt}")
                    for n in range(2):
                        ps = ps_m.tile([128, 512], F32, tag="ps_m",
                                       name="cwops")
                        for k in range(DT):
                            nc.tensor.matmul(
                                ps, aun[k][:, mt * 128:(mt + 1) * 128],
                                wo[k][:, n * 512:(n + 1) * 512],
                                start=(k == 0), stop=(k == DT - 1))
                        nc.vector.tensor_add(pre[:, n * 512:(n + 1) * 512],
                                             ps,
                                             x1[mt][:, n * 512:(n + 1) * 512])
                    nc.vector.tensor_add(pre, pre, cvec_t)
                    layer_norm(pre, g2, bt2, pre)
                    y1.append(pre)
                y1T = transposeT(y1, "y1T")

        # =================== FFN ==========================================
        with ExitStack() as ffn_ctx:
            hpool = ffn_ctx.enter_context(tc.tile_pool(name="hpool", bufs=32))
            w1pool = ffn_ctx.enter_context(tc.tile_pool(name="w1pool", bufs=8))
            with nc.named_scope("ffn1"):
                w1 = []
                for k in range(DT):
                    t = w1pool.tile([128, F], BF16, tag="w1", name=f"w1_{k}")
                    nc.sync.dma_start(out=t,
                                      in_=d_w1.ap()[k * 128:(k + 1) * 128, :])
                    w1.append(t)
                hT = []
                for m in range(NFT):
                    ps = ps_m.tile([128, CH], F32, tag="ps_m", name="f1ps")
                    for k in range(DT):
                        nc.tensor.matmul(ps, w1[k][:, m * 128:(m + 1) * 128],
                                         y1T[k], start=(k == 0),
                                         stop=(k == DT - 1))
                    h = hpool.tile([128, CH], BF16, tag="h", name=f"h{m}")
                    nc.scalar.activation(out=h, in_=ps, func=AF.Relu,
                                         bias=b1c[:, m:m + 1], scale=1.0)
                    hT.append(h)
            with nc.named_scope("ffn2"):
                b2v_t = bcast_row(d_b2v, "vec", "b2v")
                h2 = [resp.tile([128, D], F32, tag="persist", name=f"h2_{i}")
                      for i in range(NMT)]
                for n in range(2):
                    pss = [ps_s.tile([128, 2 * CH], F32, tag="ps_s",
                                     name=f"f2ps{n}_{i}") for i in range(2)]
                    for kb in range(4):
                        w2b = w1pool.tile([128, 8, 512], BF16, tag="w1",
                                          name=f"w2b{kb}")
                        nc.sync.dma_start(
                            out=w2b,
                            in_=bass.AP(tensor=d_w2.ap().tensor,
                                        offset=kb * 8 * 128 * D + n * 512,
                                        ap=[[D, 128], [128 * D, 8], [1, 512]]))
                        for ks in range(8):
                            k = kb * 8 + ks
                            for mt in range(NMT):
                                nc.tensor.matmul(
                                    pss[mt // 2][:, (mt % 2) * CH:
                                                 (mt % 2 + 1) * CH],
                                    hT[k][:, mt * 128:(mt + 1) * 128],
                                    w2b[:, ks, :],
                                    start=(k == 0), stop=(k == NFT - 1))
                    for mt in range(NMT):
                        nc.vector.tensor_add(
                            h2[mt][:, n * 512:(n + 1) * 512],
                            pss[mt // 2][:, (mt % 2) * CH:(mt % 2 + 1) * CH],
                            y1[mt][:, n * 512:(n + 1) * 512])
            with nc.named_scope("ln3_out"):
                g3 = bcast_row(d_gbt["f_g"], "gt", "g3")
                bt3 = bcast_row(d_gbt["f_bt"], "gt", "bt3")
                for mt in range(NMT):
                    nc.vector.tensor_add(h2[mt], h2[mt], b2v_t)
                    layer_norm(h2[mt], g3, bt3, h2[mt])
                    nc.sync.dma_start(out=d_out.ap()[mt * 128:(mt + 1) * 128, :],
                                      in_=h2[mt])

    nc.compile()
    return nc


def _bf(a):
    return np.ascontiguousarray(a, dtype=np.float32).astype(ml_dtypes.bfloat16)


def kernel(**inputs):
    global _CACHED
    if _CACHED is None:
        _CACHED = build()
    nc = _CACHED

    f = {k: np.asarray(v, dtype=np.float32) for k, v in inputs.items()}
    dec, enc = f["decoder_input"], f["encoder_output"]
    cvec = (f["ca_bv"] @ f["ca_wo"] + f["ca_bo"]).astype(np.float32)
    r1vec = (f["sa_bv"] @ f["sa_wo"] + f["sa_bo"]).astype(np.float32)

    shared = {n: _bf(f[n]) for n in
              ["ca_wq", "ca_wk", "ca_wv", "ca_wo", "f_w1", "f_w2"]}
    shared.update({n: f[n] for n in ["ca_bq", "ca_bk", "f_b1"]})
    shared["cvec"] = _bf(cvec)
    shared["b2v"] = _bf(f["f_b2"])
    for n in ["sa_g", "sa_bt", "ca_g", "ca_bt", "f_g", "f_bt"]:
        shared[n] = _bf(f[n])

    in_maps = []
    rows_of = {}
    for c in range(8):
        b, j = c // 4, c % 4
        rows = np.concatenate([np.arange(512 * qc + 128 * j,
                                         512 * qc + 128 * j + 128)
                               for qc in range(4)])
        rows_of[c] = rows
        hs = slice(j * 256, (j + 1) * 256)
        m = {
            "xT": _bf(dec[b].T),
            "eTq": _bf(enc[b, rows, :].T),
            "res1b": _bf(dec[b, rows, :] + r1vec),
            "sa_wq_h": _bf(f["sa_wq"][:, hs]),
            "sa_wk_h": _bf(f["sa_wk"][:, hs]),
            "sa_wv_h": _bf(f["sa_wv"][:, hs]),
            "sa_wo_h": _bf(f["sa_wo"][hs, :]),
            "sa_bq_h": np.ascontiguousarray(f["sa_bq"][hs], dtype=np.float32),
            "sa_bk_h": np.ascontiguousarray(f["sa_bk"][hs], dtype=np.float32),
        }
        m.update(shared)
        in_maps.append(m)

    global LAST_RES
    res = bass_utils.run_bass_kernel_spmd(nc, in_maps, core_ids=list(range(8)))
    LAST_RES = res
    out = np.empty((B, S, D), dtype=np.float32)
    for c in range(8):
        b = c // 4
        out[b, rows_of[c], :] = res.results[c]["out"]
    return out
